# revision 1
# baseline (speedup 1.0000x reference)
"""Trainium2 Bass kernel for nn_CoreferenceResolver (coref UNet + pair decoder).

Sharding: core c handles batch b=c//2 and pair-half h=c%2 (496 of 992 pairs).
The gather/cosine/UNet stages are replicated on the two cores sharing a batch;
the extractor linears and group-bilinear decoder are sharded over pairs.
"""
import os
import sys

for _p in ("/opt/trn_rl_repo",):
    if os.path.isdir(_p) and _p not in sys.path:
        sys.path.insert(0, _p)

import numpy as np

import concourse.bass as bass
import concourse.tile as tile
from concourse import bacc, mybir
from concourse.bass_utils import run_bass_kernel_spmd

f32 = mybir.dt.float32
i16 = mybir.dt.int16
AF = mybir.ActivationFunctionType
OP = mybir.AluOpType
f32r = mybir.dt.float32r
bf16 = mybir.dt.bfloat16


def _f(ap):
    return ap.bitcast(mybir.dt.float32)


def _r(ap):
    """View an fp32 AP as float32r for full-rate PE streaming (N>=256)."""
    return ap.bitcast(f32r)

B, L, D, H = 4, 1024, 768, 12
NE, P = 32, 992
BLOCK = 64
G = D // BLOCK          # 12 groups
OUT_CH = 256
NCORES = 8
NH = P // 2             # 496 pairs per core
KD = D // 128           # 6 chunks of the D dim


def build_nc():
    nc = bacc.Bacc("TRN2", target_bir_lowering=False, debug=False, num_devices=NCORES)

    def inp(name, shape, dt=f32):
        return nc.dram_tensor(name, shape, dt, kind="ExternalInput")

    x_b      = inp("x_b", [L, D])
    ent_idx  = inp("ent_idx", [128, 2], i16)
    ent_mask = inp("ent_mask", [NE, 1])
    iota32   = inp("iota32", [NE, 1])
    ident    = inp("ident", [NE, NE])
    smat     = inp("smat", [128, 2], bf16)
    ones_r   = inp("ones_r", [1, 128], f32r)
    hi_f     = inp("hi_f", [1, NH])
    ti_f     = inp("ti_f", [1, NH])
    pair_idx = inp("pair_idx", [128, NH // 16], i16)

    enc1_w9  = inp("enc1_w9", [1, 9 * 64], f32r);        enc1_bp = inp("enc1_bp", [64, 1])
    enc2_w9  = inp("enc2_w9", [64, 9, 128], f32r);   enc2_bp = inp("enc2_bp", [128, 1])
    bott_w9  = inp("bott_w9", [128, 9, 256], f32r);  bott_bp = inp("bott_bp", [128, 2])
    ag2_wgp  = inp("ag2_wgp", [128, 2, 128], f32r)
    ag2_wxp  = inp("ag2_wxp", [128, 128], f32r)
    ag2_psip = inp("ag2_psip", [128, 1], f32r)
    dec2_w9  = inp("dec2_w9", [128, 3, 9, 128], f32r); dec2_bp = inp("dec2_bp", [128, 1])
    ag1_wgp  = inp("ag1_wgp", [128, 64], f32r)
    ag1_wxp  = inp("ag1_wxp", [64, 64], f32r)
    ag1_psip = inp("ag1_psip", [64, 1], f32r)
    dec1_w9a = inp("dec1_w9a", [128, 9, 64], f32r)
    dec1_w9b = inp("dec1_w9b", [64, 9, 64], f32r);   dec1_bp = inp("dec1_bp", [64, 1])
    fin_wp   = inp("fin_wp", [64, 256], f32r);       fin_bp  = inp("fin_bp", [128, 2])

    W1h = inp("W1h", [128, KD, D], f32r)   # head_w[:768] K-chunked
    W2h = inp("W2h", [128, 2, D], f32r)    # head_w[768:] K-chunked
    W1t = inp("W1t", [128, KD, D], f32r)
    W2t = inp("W2t", [128, 2, D], f32r)
    head_bp = inp("head_bp", [128, KD])
    tail_bp = inp("tail_bp", [128, KD])
    wdec = inp("wdec", [128, G, 128], f32r)   # rows 0:64 == rows 64:128 (host-duplicated)
    dec_bp = inp("dec_bp", [2, 1])

    y = nc.dram_tensor("y", [2, NH], f32, kind="ExternalOutput")

    from contextlib import ExitStack
    with tile.TileContext(nc) as tc, ExitStack() as _ctx:
        sbw = _ctx.enter_context(tc.tile_pool(name="sbw", bufs=1))   # persistent
        sbt = _ctx.enter_context(tc.tile_pool(name="sbt", bufs=3))   # rotating temps
        sws = _ctx.enter_context(tc.tile_pool(name="sws", bufs=4))   # streamed W1 chunks

        # ---------------- load persistent tensors ----------------
        def load(t, shape, dt=f32, name=None, early=False):
            tt = sbw.tile(shape, dt, tag=name or t.name)
            (nc.gpsimd if early else nc.sync).dma_start(tt[:], t[:])
            return tt

        t_eidx  = load(ent_idx, [128, 2], i16, "eidx", early=True)
        t_emask = load(ent_mask, [NE, 1], f32, "emask")
        t_iota  = load(iota32, [NE, 1], f32, "iota")
        t_ident = load(ident, [NE, NE], f32, "ident")
        t_smat  = load(smat, [128, 2], bf16, "smat")
        t_ones  = load(ones_r, [1, 128], f32r, "ones")
        t_hif   = load(hi_f, [1, NH], f32, "hif")
        t_tif   = load(ti_f, [1, NH], f32, "tif")
        pu_cm = tc.tile_pool(name="pu", bufs=3, space="PSUM")
        pu = pu_cm.__enter__()

        # ---------------- entity gather + normalize ----------------
        nrm   = sbw.tile([NE, D], f32, tag="nrm")
        nrmT  = sbw.tile([128, KD, NE], f32, tag="nrmT")
        normc = sbw.tile([NE, 1], f32, tag="normc")
        ent_raw = sbt.tile([128, 1, D], f32, tag="entraw")
        nc.gpsimd.dma_gather(ent_raw[:], x_b[:], t_eidx[:],
                             num_idxs=NE, num_idxs_reg=NE, elem_size=D)
        ent = ent_raw[0:NE, 0, :]
        sq = sbt.tile([NE, D], f32, tag="t")
        nc.vector.tensor_mul(sq[:], ent, ent)
        ss = sbt.tile([NE, 1], f32, tag="ss")
        nc.vector.reduce_sum(ss[:], sq[:], axis=mybir.AxisListType.X)
        nc.scalar.sqrt(normc[:], ss[:])
        nc.vector.tensor_single_scalar(normc[:], normc[:], 1e-13, op=OP.max)
        rinv = sbt.tile([NE, 1], f32, tag="rinv")
        nc.vector.reciprocal(rinv[:], normc[:])
        nc.vector.tensor_tensor(out=rinv[:], in0=rinv[:], in1=t_emask[:], op=OP.mult)
        nc.vector.tensor_scalar(out=nrm[:], in0=ent, scalar1=rinv[:],
                                scalar2=None, op0=OP.mult)
        for k in range(KD):
            p_t = pu.tile([128, NE], f32, tag="pu")
            nc.tensor.transpose(p_t[:], nrm[:, k * 128:(k + 1) * 128], t_ident[:])
            nc.vector.tensor_copy(_r(nrmT[:, k, :]), p_t[:])

        t_pidx  = load(pair_idx, [128, NH // 16], i16, "pidx")

        t_enc1w = load(enc1_w9, [1, 9 * 64], f32r, "enc1w")
        t_enc1b = load(enc1_bp, [64, 1], f32, "enc1b")
        t_enc2w = load(enc2_w9, [64, 9, 128], f32r, "enc2w")
        t_enc2b = load(enc2_bp, [128, 1], f32, "enc2b")
        t_bottw = load(bott_w9, [128, 9, 256], f32r, "bottw")
        t_bottb = load(bott_bp, [128, 2], f32, "bottb")
        t_ag2wg = load(ag2_wgp, [128, 2, 128], f32r, "ag2wg")
        t_ag2wx = load(ag2_wxp, [128, 128], f32r, "ag2wx")
        t_ag2ps = load(ag2_psip, [128, 1], f32r, "ag2ps")
        t_dec2w = load(dec2_w9, [128, 3, 9, 128], f32r, "dec2w")
        t_dec2b = load(dec2_bp, [128, 1], f32, "dec2b")
        t_ag1wg = load(ag1_wgp, [128, 64], f32r, "ag1wg")
        t_ag1wx = load(ag1_wxp, [64, 64], f32r, "ag1wx")
        t_ag1ps = load(ag1_psip, [64, 1], f32r, "ag1ps")
        t_dec1wa = load(dec1_w9a, [128, 9, 64], f32r, "dec1wa")
        t_dec1wb = load(dec1_w9b, [64, 9, 64], f32r, "dec1wb")
        t_dec1b = load(dec1_bp, [64, 1], f32, "dec1b")
        t_finw  = load(fin_wp, [64, 256], f32r, "finw")
        t_finb  = load(fin_bp, [128, 2], f32, "finb")
        t_w2h   = load(W2h, [128, 2, D], f32r, "w2h")
        t_w2t   = load(W2t, [128, 2, D], f32r, "w2t")
        t_hbp   = load(head_bp, [128, KD], f32, "hbp")
        t_tbp   = load(tail_bp, [128, KD], f32, "tbp")
        t_wdec  = load(wdec, [128, G, 128], f32r, "wdec")
        t_decb  = load(dec_bp, [2, 1], f32, "decb")

        # ---------------- persistent intermediates ----------------
        img0  = sbw.tile([1, 34 * 34], f32, tag="img0")
        c1p   = sbw.tile([64, 34 * 34], f32, tag="c1p")
        p1p   = sbw.tile([64, 18 * 18], f32, tag="p1p")
        c2p   = sbw.tile([128, 18 * 18], f32, tag="c2p")
        p2p   = sbw.tile([128, 10 * 10], f32, tag="p2p")
        u2p0  = sbw.tile([128, 18 * 18], f32, tag="u2p0")
        u2p1  = sbw.tile([128, 18 * 18], f32, tag="u2p1")
        att2p = sbw.tile([128, 18 * 18], f32, tag="att2p")
        d2s   = sbw.tile([128, 256], f32, tag="d2s")
        u1p   = sbw.tile([128, 34 * 34], f32, tag="u1p")
        att1p = sbw.tile([64, 34 * 34], f32, tag="att1p")
        d1s   = sbw.tile([64, 1024], f32, tag="d1s")
        amap0 = sbw.tile([128, 1024], f32, tag="amap0")
        amap1 = sbw.tile([128, 1024], f32, tag="amap1")

        ew1   = sbw.tile([NE, D], f32, tag="ew1")
        et1   = sbw.tile([NE, D], f32, tag="et1")
        ohhi  = sbw.tile([NE, NH], f32, tag="ohhi")
        ohti  = sbw.tile([NE, NH], f32, tag="ohti")
        htT0  = sbw.tile([128, NH], f32, tag="htT0")
        htT1  = sbw.tile([128, NH], f32, tag="htT1")
        hsT   = sbw.tile([128, KD, NH], f32, tag="hsT")
        tsT   = sbw.tile([128, KD, NH], f32, tag="tsT")

        # zero the padded borders once (rounded writes: the borders feed f32r matmuls)
        for t in (img0, c1p, p1p, c2p, p2p, u2p0, u2p1, att2p, u1p, att1p):
            nc.gpsimd.memset(t[:], 0.0)

        # ---------------- cosine matrix ----------------
        p_cos = pu.tile([NE, NE], f32, tag="pu")
        for k in range(KD):
            nc.tensor.matmul(p_cos[:], nrmT[:, k, :], nrmT[:, k, :],
                             start=(k == 0), stop=(k == KD - 1))
        s_cos = sbt.tile([NE, NE], f32, tag="scos")
        nc.vector.tensor_copy(_r(s_cos[:]), p_cos[:])

        # ---------------- UNet ----------------
        # enc1: one padded image (DMA issued from DVE right after the cos
        # copy - no cross-queue hop), then 9 taps x 2 halves of K=1 matmuls
        img0v = img0[:].rearrange("c (h w) -> c h w", h=34, w=34)
        nc.gpsimd.dma_start(_r(img0v[0:1, 1:33, 1:33]), _r(s_cos[:]))
        p_c1 = pu.tile([64, 1024], f32, tag="pu")
        for hh in range(2):
            n_mm = 0
            for tap in range(9):
                dy, dx = tap // 3, tap % 3
                rows = slice(dy + 16 * hh, dy + 16 * hh + 16)
                nc.tensor.matmul(p_c1[:, hh * 512:(hh + 1) * 512],
                                 t_enc1w[0:1, tap * 64:(tap + 1) * 64],
                                 _r(img0v[0:1, rows, dx:dx + 32]),
                                 start=(n_mm == 0), stop=(n_mm == 8))
                n_mm += 1
        c1pv = c1p[:].rearrange("c (h w) -> c h w", h=34, w=34)
        for hh in range(2):
            nc.scalar.activation(_r(c1pv[:, 1 + 16 * hh:17 + 16 * hh, 1:33]),
                                 p_c1[:, hh * 512:(hh + 1) * 512].rearrange(
                                     "c (h w) -> c h w", h=16, w=32),
                                 AF.Relu, bias=t_enc1b[:])

        # pool1 -> p1p interior [64, 16, 16]
        p1pv = p1p[:].rearrange("c (h w) -> c h w", h=18, w=18)
        tmp = sbt.tile([64, 16, 16], f32, tag="t")
        nc.vector.tensor_max(tmp[:], c1pv[:, 1:33:2, 1:33:2], c1pv[:, 1:33:2, 2:34:2])
        nc.vector.tensor_max(tmp[:], tmp[:], c1pv[:, 2:34:2, 1:33:2])
        nc.vector.tensor_max(_r(p1pv[:, 1:17, 1:17]), tmp[:], c1pv[:, 2:34:2, 2:34:2])

        # enc2: 9 shifted matmuls K=64
        p_c2 = pu.tile([128, 256], f32, tag="pu")
        for tap in range(9):
            dy, dx = tap // 3, tap % 3
            nc.tensor.matmul(p_c2[:], _r(t_enc2w[:, tap, :]),
                             _r(p1pv[:, dy:dy + 16, dx:dx + 16]),
                             start=(tap == 0), stop=(tap == 8))
        c2pv = c2p[:].rearrange("c (h w) -> c h w", h=18, w=18)
        nc.scalar.activation(_r(c2pv[:, 1:17, 1:17]),
                             p_c2[:].rearrange("c (h w) -> c h w", h=16, w=16),
                             AF.Relu, bias=t_enc2b[:])

        # pool2 -> p2p interior [128, 8, 8]
        p2pv = p2p[:].rearrange("c (h w) -> c h w", h=10, w=10)
        tmp2 = sbt.tile([128, 8, 8], f32, tag="t")
        nc.vector.tensor_max(tmp2[:], c2pv[:, 1:17:2, 1:17:2], c2pv[:, 1:17:2, 2:18:2])
        nc.vector.tensor_max(tmp2[:], tmp2[:], c2pv[:, 2:18:2, 1:17:2])
        nc.vector.tensor_max(_r(p2pv[:, 1:9, 1:9]), tmp2[:], c2pv[:, 2:18:2, 2:18:2])

        # bottleneck: 9 taps x 2 M-chunks, K=128
        c3 = []
        for mc in range(2):
            p_c3 = pu.tile([128, 64], f32, tag="pu")
            for tap in range(9):
                dy, dx = tap // 3, tap % 3
                nc.tensor.matmul(p_c3[:], t_bottw[:, tap, mc * 128:(mc + 1) * 128],
                                 _r(p2pv[:, dy:dy + 8, dx:dx + 8]),
                                 start=(tap == 0), stop=(tap == 8))
            c3s = sbt.tile([128, 8, 8], f32, tag=f"c3_{mc}")
            nc.scalar.activation(c3s[:], p_c3[:].rearrange("c (h w) -> c h w", h=8, w=8),
                                 AF.Relu, bias=t_bottb[:, mc:mc + 1])
            c3.append(c3s)

        # up2 -> u2p interior [128, 16, 16] x2 chunks
        for mc, (src, dst) in enumerate(((c3[0], u2p0), (c3[1], u2p1))):
            dv = dst[:].rearrange("c (h w) -> c h w", h=18, w=18)
            for i in range(2):
                for j in range(2):
                    nc.vector.tensor_copy(_r(dv[:, 1 + i:17:2, 1 + j:17:2]), src[:])

        u2p0v = u2p0[:].rearrange("c (h w) -> c h w", h=18, w=18)
        u2p1v = u2p1[:].rearrange("c (h w) -> c h w", h=18, w=18)

        # attention gate 2: relu(wg@u2 + wx@c2) -> psi -> sigmoid -> c2*a
        p_a2 = pu.tile([128, 256], f32, tag="pu")
        nc.tensor.matmul(p_a2[:], _r(t_ag2wg[:, 0, :]), _r(u2p0v[:, 1:17, 1:17]),
                         start=True, stop=False)
        nc.tensor.matmul(p_a2[:], _r(t_ag2wg[:, 1, :]), _r(u2p1v[:, 1:17, 1:17]),
                         start=False, stop=False)
        nc.tensor.matmul(p_a2[:], _r(t_ag2wx[:]), _r(c2pv[:, 1:17, 1:17]),
                         start=False, stop=True)
        r2 = sbt.tile([128, 256], f32, tag="t")
        nc.scalar.activation(_r(r2[:]), p_a2[:], AF.Relu)
        p_g2 = pu.tile([1, 256], f32, tag="pu")
        nc.tensor.matmul(p_g2[:], t_ag2ps[:], _r(r2[:]))
        a2 = sbt.tile([1, 256], f32, tag="a2")
        nc.scalar.activation(_r(a2[:]), p_g2[:], AF.Sigmoid)
        p_a2b = pu.tile([128, 256], f32, tag="pu")
        nc.tensor.matmul(p_a2b[:], t_ones[:], _r(a2[:]))
        att2pv = att2p[:].rearrange("c (h w) -> c h w", h=18, w=18)
        att2t = sbt.tile([128, 256], f32, tag="t")
        nc.vector.tensor_mul(att2t[:].rearrange("c (h w) -> c h w", h=16, w=16),
                             p_a2b[:].rearrange("c (h w) -> c h w", h=16, w=16),
                             c2pv[:, 1:17, 1:17])
        nc.vector.tensor_copy(_r(att2pv[:, 1:17, 1:17]),
                              att2t[:].rearrange("c (h w) -> c h w", h=16, w=16))

        # dec2: 9 taps x 3 K-chunks (u2p0, u2p1, att2p)
        p_d2 = pu.tile([128, 256], f32, tag="pu")
        srcs2 = (u2p0v, u2p1v, att2pv)
        n_mm = 0
        for tap in range(9):
            dy, dx = tap // 3, tap % 3
            for kc in range(3):
                nc.tensor.matmul(p_d2[:], _r(t_dec2w[:, kc, tap, :]),
                                 _r(srcs2[kc][:, dy:dy + 16, dx:dx + 16]),
                                 start=(n_mm == 0), stop=(n_mm == 26))
                n_mm += 1
        nc.scalar.activation(d2s[:], p_d2[:], AF.Relu, bias=t_dec2b[:])

        # up1 -> u1p interior [128, 32, 32]
        u1pv = u1p[:].rearrange("c (h w) -> c h w", h=34, w=34)
        d2v = d2s[:].rearrange("c (h w) -> c h w", h=16, w=16)
        for i in range(2):
            for j in range(2):
                nc.vector.tensor_copy(_r(u1pv[:, 1 + i:33:2, 1 + j:33:2]), d2v[:])

        # attention gate 1
        p_a1 = pu.tile([64, 1024], f32, tag="pu")
        for hh in range(2):
            rows = slice(1 + 16 * hh, 17 + 16 * hh)
            nc.tensor.matmul(p_a1[:, hh * 512:(hh + 1) * 512], _r(t_ag1wg[:]),
                             _r(u1pv[:, rows, 1:33]), start=True, stop=False)
            nc.tensor.matmul(p_a1[:, hh * 512:(hh + 1) * 512], _r(t_ag1wx[:]),
                             _r(c1pv[:, rows, 1:33]), start=False, stop=True)
        r1 = sbt.tile([64, 1024], f32, tag="t")
        nc.scalar.activation(_r(r1[:]), p_a1[:], AF.Relu)
        p_g1 = pu.tile([1, 1024], f32, tag="pu")
        for hh in range(2):
            nc.tensor.matmul(p_g1[:, hh * 512:(hh + 1) * 512], t_ag1ps[:],
                             _r(r1[:, hh * 512:(hh + 1) * 512]))
        a1 = sbt.tile([1, 1024], f32, tag="a1")
        nc.scalar.activation(_r(a1[:]), p_g1[:], AF.Sigmoid)
        p_a1b = pu.tile([64, 1024], f32, tag="pu")
        for hh in range(2):
            nc.tensor.matmul(p_a1b[:, hh * 512:(hh + 1) * 512], t_ones[:, :64],
                             _r(a1[:, hh * 512:(hh + 1) * 512]))
        att1pv = att1p[:].rearrange("c (h w) -> c h w", h=34, w=34)
        att1t = sbt.tile([64, 1024], f32, tag="t")
        nc.vector.tensor_mul(att1t[:].rearrange("c (h w) -> c h w", h=32, w=32),
                             p_a1b[:].rearrange("c (h w) -> c h w", h=32, w=32),
                             c1pv[:, 1:33, 1:33])
        nc.vector.tensor_copy(_r(att1pv[:, 1:33, 1:33]),
                              att1t[:].rearrange("c (h w) -> c h w", h=32, w=32))

        # dec1: 9 taps x (u1p K=128 + att1p K=64) x 2 N-halves
        p_d1 = pu.tile([64, 1024], f32, tag="pu")
        for hh in range(2):
            n_mm = 0
            for tap in range(9):
                dy, dx = tap // 3, tap % 3
                rows = slice(dy + 16 * hh, dy + 16 * hh + 16)
                nc.tensor.matmul(p_d1[:, hh * 512:(hh + 1) * 512],
                                 _r(t_dec1wa[:, tap, :]), _r(u1pv[:, rows, dx:dx + 32]),
                                 start=(n_mm == 0), stop=False)
                n_mm += 1
                nc.tensor.matmul(p_d1[:, hh * 512:(hh + 1) * 512],
                                 _r(t_dec1wb[:, tap, :]), _r(att1pv[:, rows, dx:dx + 32]),
                                 start=False, stop=(n_mm == 17))
                n_mm += 1
            nc.scalar.activation(_r(d1s[:, hh * 512:(hh + 1) * 512]),
                                 p_d1[:, hh * 512:(hh + 1) * 512],
                                 AF.Relu, bias=t_dec1b[:])

        # fin 1x1 conv -> amapT [256, 1024] in two chunks (with bias, no relu)
        for mc, dst in ((0, amap0), (1, amap1)):
            p_am = pu.tile([128, 1024], f32, tag="pu")
            for hh in range(2):
                nc.tensor.matmul(p_am[:, hh * 512:(hh + 1) * 512],
                                 _r(t_finw[:, mc * 128:(mc + 1) * 128]),
                                 _r(d1s[:, hh * 512:(hh + 1) * 512]))
            nc.scalar.activation(dst[:], p_am[:], AF.Identity, bias=t_finb[:, mc:mc + 1])

        # ---------------- extractor premultiplies ----------------
        # EW1 = ent @ head_w[:768]  (= maxnorm-scaled nrm @ W1), same for tail
        for (wsrc, dst) in ((W1h, ew1), (W1t, et1)):
            p_ew = pu.tile([NE, D], f32, tag="pu")
            for k in range(KD):
                wchunk = sws.tile([128, D], f32r, tag="wbig")
                nc.sync.dma_start(wchunk[:], wsrc[:, k, :])
                for n0, n1 in ((0, 512), (512, 768)):
                    nc.tensor.matmul(p_ew[:, n0:n1],
                                     _r(nrmT[:, k, :]), _r(wchunk[:, n0:n1]),
                                     start=(k == 0), stop=(k == KD - 1))
            nc.scalar.activation(_r(dst[:]), p_ew[:], AF.Copy, scale=normc[:])

        # one-hot selector matrices for hi / ti
        for (src, dst) in ((t_hif, ohhi), (t_tif, ohti)):
            bc = sbt.tile([NE, NH], f32, tag="t")
            nc.gpsimd.partition_broadcast(bc[:], src[:])
            nc.vector.tensor_scalar(out=_r(dst[:]), in0=bc[:], scalar1=t_iota[:],
                                    scalar2=None, op0=OP.is_equal)

        # gather amap columns for each pair: htT = amapT[:, pair_idx]
        htT0x = sbt.tile([128, NH], f32, tag="t")
        htT1x = sbt.tile([128, NH], f32, tag="t")
        nc.gpsimd.ap_gather(htT0x[:].rearrange("c (n o) -> c n o", o=1),
                            amap0[:].rearrange("c (n o) -> c n o", o=1), t_pidx[:],
                            channels=128, num_elems=1024, d=1, num_idxs=NH)
        nc.gpsimd.ap_gather(htT1x[:].rearrange("c (n o) -> c n o", o=1),
                            amap1[:].rearrange("c (n o) -> c n o", o=1), t_pidx[:],
                            channels=128, num_elems=1024, d=1, num_idxs=NH)
        nc.vector.tensor_copy(_r(htT0[:]), htT0x[:])
        nc.vector.tensor_copy(_r(htT1[:]), htT1x[:])

        pu_cm.__exit__(None, None, None)

        # ---------------- pair features + decoder, interleaved per chunk ----
        # for each of the 6 D-chunks: head tanh-arg, tail tanh-arg, then the
        # two decoder groups of that chunk - keeps PE/ACT/DVE pipelined
        ph_cm = tc.tile_pool(name="ph", bufs=4, space="PSUM")
        ph = ph_cm.__enter__()
        pd_cm = tc.tile_pool(name="pd", bufs=2, space="PSUM")
        pd = pd_cm.__enter__()
        po_cm = tc.tile_pool(name="po", bufs=1, space="PSUM")
        po = po_cm.__enter__()
        p_out = po.tile([2, NH], f32, tag="po")
        for k in range(KD):
            cols = slice(k * 128, (k + 1) * 128)
            for (ewt, oh, w2, bp, dstT) in ((ew1, ohhi, t_w2h, t_hbp, hsT),
                                            (et1, ohti, t_w2t, t_tbp, tsT)):
                p_hs = ph.tile([128, NH], f32, tag="ph")
                nc.tensor.matmul(p_hs[:], _r(ewt[:, cols]), _r(oh[:]), start=True, stop=False)
                nc.tensor.matmul(p_hs[:], _r(w2[:, 0, cols]), _r(htT0[:]), start=False, stop=False)
                nc.tensor.matmul(p_hs[:], _r(w2[:, 1, cols]), _r(htT1[:]), start=False, stop=True)
                nc.scalar.activation(_r(dstT[:, k, :]), p_hs[:],
                                     AF.Tanh, bias=bp[:, k:k + 1])
            for half in range(2):
                g = 2 * k + half
                rows = slice(half * 64, (half + 1) * 64)
                p_u = pd.tile([128, NH], f32, tag="pd")
                nc.tensor.matmul(p_u[:], _r(t_wdec[rows, g, :]), _r(tsT[rows, k, :]))
                v = sbt.tile([128, NH], bf16, tag="v")
                nc.vector.tensor_mul(v[0:64, :], p_u[0:64, :], hsT[rows, k, :])
                nc.vector.tensor_mul(v[64:128, :], p_u[64:128, :], hsT[rows, k, :])
                nc.tensor.matmul(p_out[:], t_smat[:], v[:],
                                 start=(g == 0), stop=(g == G - 1))
        out_sb = sbt.tile([2, NH], f32, tag="out")
        nc.scalar.activation(out_sb[:], p_out[:], AF.Identity, bias=t_decb[:])
        nc.sync.dma_start(y[:], out_sb[:])
        po_cm.__exit__(None, None, None)
        pd_cm.__exit__(None, None, None)
        ph_cm.__exit__(None, None, None)

    nc.compile()
    return nc


def f32r_round(a):
    """Round-to-nearest-even to fp32r (11 mantissa bits), matching the PE."""
    u = np.ascontiguousarray(a, np.float32).view(np.uint32).copy()
    u = (u + (np.uint32(0x7FF) + ((u >> np.uint32(12)) & np.uint32(1)))) & np.uint32(0xFFFFF000)
    return u.view(np.float32)


def _wrap16(idx, n_slots):
    """int16 index layout for gpsimd gathers: wrapped in 16 partitions,
    replicated across the 8 gpsimd cores."""
    out = np.zeros((128, n_slots), np.int16)
    for j, v in enumerate(idx):
        out[np.arange(8) * 16 + j % 16, j // 16] = v
    return out


def pack_inputs(inputs):
    """Build the 8 per-core input maps from the full problem inputs."""
    x = np.asarray(inputs["x"], np.float32)
    entity_pos = np.asarray(inputs["entity_pos"])
    hts = np.asarray(inputs["hts"])

    shared = {}
    shared["iota32"] = np.arange(NE, dtype=np.float32).reshape(NE, 1)
    shared["ident"] = np.eye(NE, dtype=np.float32)
    smat = np.zeros((128, 2), np.float32)
    smat[:64, 0] = 1.0
    smat[64:, 1] = 1.0
    shared["smat"] = smat  # cast below
    shared["ones_r"] = np.ones((1, 128), np.float32)

    def W(name):
        return np.asarray(inputs[name], np.float32)

    shared["enc1_w9"] = W("enc1_w").reshape(64, 9).T.reshape(1, 576).copy()
    shared["enc1_bp"] = W("enc1_b").reshape(64, 1)
    shared["enc2_w9"] = W("enc2_w").reshape(128, 64, 9).transpose(1, 2, 0).copy()
    shared["enc2_bp"] = W("enc2_b").reshape(128, 1)
    shared["bott_w9"] = W("bott_w").reshape(256, 128, 9).transpose(1, 2, 0).copy()
    shared["bott_bp"] = W("bott_b").reshape(2, 128).T.copy()
    shared["ag2_wgp"] = W("ag2_wg").reshape(128, 256).T.reshape(2, 128, 128).transpose(1, 0, 2).copy()
    shared["ag2_wxp"] = W("ag2_wx").reshape(128, 128).T.copy()
    shared["ag2_psip"] = W("ag2_psi").reshape(1, 128).T.copy()
    shared["dec2_w9"] = W("dec2_w").reshape(128, 384, 9).transpose(1, 2, 0).reshape(3, 128, 9, 128).transpose(1, 0, 2, 3).copy()
    shared["dec2_bp"] = W("dec2_b").reshape(128, 1)
    shared["ag1_wgp"] = W("ag1_wg").reshape(64, 128).T.copy()
    shared["ag1_wxp"] = W("ag1_wx").reshape(64, 64).T.copy()
    shared["ag1_psip"] = W("ag1_psi").reshape(1, 64).T.copy()
    d1w = W("dec1_w").reshape(64, 192, 9).transpose(1, 2, 0)   # [192, 9, 64]
    shared["dec1_w9a"] = d1w[:128].copy()
    shared["dec1_w9b"] = d1w[128:].copy()
    shared["dec1_bp"] = W("dec1_b").reshape(64, 1)
    shared["fin_wp"] = W("fin_w").reshape(256, 64).T.copy()
    shared["fin_bp"] = W("fin_b").reshape(2, 128).T.copy()

    head_w = W("head_w"); tail_w = W("tail_w")
    shared["W1h"] = head_w[:D].reshape(KD, 128, D).transpose(1, 0, 2).copy()
    shared["W2h"] = head_w[D:].reshape(2, 128, D).transpose(1, 0, 2).copy()
    shared["W1t"] = tail_w[:D].reshape(KD, 128, D).transpose(1, 0, 2).copy()
    shared["W2t"] = tail_w[D:].reshape(2, 128, D).transpose(1, 0, 2).copy()
    shared["head_bp"] = W("head_b").reshape(KD, 128).T.copy()
    shared["tail_bp"] = W("tail_b").reshape(KD, 128).T.copy()
    wd = W("decoder_w").reshape(G, 64, 64, 2).transpose(2, 0, 3, 1).reshape(64, G, 128)
    shared["wdec"] = np.concatenate([wd, wd], axis=0).copy()   # rows duplicated
    shared["dec_bp"] = W("decoder_b").reshape(2, 1)

    for k in ("enc1_w9", "enc2_w9", "bott_w9", "ag2_wgp", "ag2_wxp", "ag2_psip",
              "dec2_w9", "ag1_wgp", "ag1_wxp", "ag1_psip", "dec1_w9a", "dec1_w9b",
              "fin_wp", "W1h", "W2h", "W1t", "W2t", "wdec"):
        shared[k] = f32r_round(shared[k])
    import ml_dtypes
    shared["smat"] = shared["smat"].astype(ml_dtypes.bfloat16)

    in_maps = []
    for c in range(NCORES):
        b, h = c // 2, c % 2
        m = dict(shared)
        m["x_b"] = np.ascontiguousarray(x[b])
        start = entity_pos[b, :, 0].astype(np.int64)
        idx = np.minimum(start + 1, L - 1).astype(np.int16)
        m["ent_idx"] = _wrap16(idx, 2)
        m["ent_mask"] = (start + 1 < L).astype(np.float32).reshape(NE, 1)
        hi = hts[b, h * NH:(h + 1) * NH, 0].astype(np.int64)
        ti = hts[b, h * NH:(h + 1) * NH, 1].astype(np.int64)
        m["hi_f"] = hi.astype(np.float32).reshape(1, NH)
        m["ti_f"] = ti.astype(np.float32).reshape(1, NH)
        m["pair_idx"] = _wrap16((hi * NE + ti).astype(np.int16), NH // 16)
        in_maps.append(m)
    return in_maps


_NC_CACHE = None


def get_nc():
    global _NC_CACHE
    if _NC_CACHE is None:
        _NC_CACHE = build_nc()
    return _NC_CACHE


def kernel(**inputs):
    nc = get_nc()
    in_maps = pack_inputs(inputs)
    res = run_bass_kernel_spmd(nc, in_maps, core_ids=list(range(NCORES)))
    out = np.empty((B * P, 2), np.float32)
    for c in range(NCORES):
        b, h = c // 2, c % 2
        yc = res.results[c]["y"]                  # [2, NH]
        out[b * P + h * NH:b * P + (h + 1) * NH, :] = yc.T
    return out



# revision 5
# speedup vs baseline: 1.2617x; 1.2617x over previous
"""Trainium2 Bass kernel for nn_CoreferenceResolver (coref UNet + pair decoder).

Sharding: core c handles batch b=c//2 and pair-half h=c%2 (496 of 992 pairs).
The gather/cosine/UNet stages are replicated on the two cores sharing a batch;
the extractor linears and group-bilinear decoder are sharded over pairs.

v1 design notes (vs the f32r baseline):
- all weights/activations bf16 (halves DMA bytes, PE still 1 cycle/row)
- fin 1x1 conv + amap gathers + W2 matmuls folded into host-precomputed
  W2' = fin_w^T @ head_w[768:] and a single d1 gather
- extractor = one stacked K=128 matmul per (k, extractor):
  mov rows 0:64 d1[pairs], 64:96 one-hot(hi) (host), 96:128 one-hot(ti)
- enc1 via 3 column-shifted padded images (K=3 matmuls, 6 total)
- weights arrive as a few packed DMA chunks ordered just-in-time so the
  cos->image DMA never queues behind megabytes of weight traffic
- PE warmup chain holds the p-state ramp so real matmuls price at full speed
"""
import os
import sys

for _p in ("/opt/trn_rl_repo",):
    if os.path.isdir(_p) and _p not in sys.path:
        sys.path.insert(0, _p)

import numpy as np

import concourse.bass as bass
import concourse.tile as tile
from concourse import bacc, mybir
from concourse.bass_utils import run_bass_kernel_spmd

f32 = mybir.dt.float32
i16 = mybir.dt.int16
bf16 = mybir.dt.bfloat16
AF = mybir.ActivationFunctionType
OP = mybir.AluOpType

B, L, D, H = 4, 1024, 768, 12
NE, P = 32, 992
BLOCK = 64
G = D // BLOCK          # 12 groups
OUT_CH = 256
NCORES = 8
NH = P // 2             # 496 pairs per core
KD = D // 128           # 6 chunks of the D dim

# ---------------------------------------------------------------------------
# packed-chunk layouts (shapes only; shared by build_nc and pack_inputs)
# entries: name -> (row0, rows, col0, cols); each chunk = one DRAM tensor.
# ---------------------------------------------------------------------------


def _mklayout(rows, entries):
    lay, col = {}, 0
    for name, r0, r, c in entries:
        lay[name] = (r0, r, col, c)
        col += c
    return lay, col


LAY_A64, NC_A64 = _mklayout(64, [
    ("ident", 0, 32, 32),
    ("enc1w3", 0, 3, 192),          # [dx, dy*64+c]
    ("enc2w", 0, 64, 1152),         # [64, 9*128]
    ("dec1wb", 0, 64, 576),         # [64, 9*64]
    ("ag1wx", 0, 64, 64),
    ("ag1ps", 0, 64, 1),
])
LAY_BOTT, NC_BOTT = _mklayout(128, [("bott", 0, 128, 2304)])   # [128, 9*256]
LAY_AG2, NC_AG2 = _mklayout(128, [
    ("ag2wg", 0, 128, 256),         # [128, 2*128]
    ("ag2wx", 0, 128, 128),
    ("ag2ps", 0, 128, 1),
])
LAY_DEC2A, NC_DEC2A = _mklayout(128, [("dec2a", 0, 128, 2304)])  # kc 0:2
LAY_DEC2B, NC_DEC2B = _mklayout(128, [("dec2b", 0, 128, 1152)])  # kc 2
LAY_B128, NC_B128 = _mklayout(128, [
    ("dec1wa", 0, 128, 576),        # [128, 9*64]
    ("ag1wg", 0, 128, 64),
])
LAY_F, NC_F = _mklayout(128, [
    ("stk_h", 0, 128, 768),         # rows 0:64 W2h'; 64:96 <- EW1 (device)
    ("stk_t", 0, 128, 768),         # rows 0:64 W2t'; 64:96 <- EW1t (device)
    ("wdec", 0, 128, 1536),         # [128, 12*128] (rows duplicated)
    ("smat", 0, 128, 2),
])
LAY_MF, NC_MF = _mklayout(128, [
    ("emask", 0, 32, 1),
    ("enc1b", 0, 64, 1),
    ("enc2b", 0, 128, 1),
    ("bottb", 0, 128, 2),
    ("dec2b_b", 0, 128, 1),
    ("dec1b", 0, 64, 1),
    ("hbp", 0, 128, 6),
    ("tbp", 0, 128, 6),
    ("decb", 0, 2, 1),
])


def build_nc():
    nc = bacc.Bacc("TRN2", target_bir_lowering=False, debug=False, num_devices=NCORES)

    def inp(name, shape, dt=f32):
        return nc.dram_tensor(name, shape, dt, kind="ExternalInput")

    x_b = inp("x_b", [L, D])
    eidx_d = inp("eidx", [128, 8], i16)
    pidx_d = inp("pidx", [128, NH // 16], i16)
    ohm_d = inp("ohm", [64, NH], bf16)
    mf_d = inp("mf", [128, NC_MF])
    a64_d = inp("a64", [64, NC_A64], bf16)
    bott_d = inp("bott_c", [128, NC_BOTT], bf16)
    ag2_d = inp("ag2_c", [128, NC_AG2], bf16)
    dec2a_d = inp("dec2a_c", [128, NC_DEC2A], bf16)
    dec2b_d = inp("dec2b_c", [128, NC_DEC2B], bf16)
    b128_d = inp("b128", [128, NC_B128], bf16)
    w1h_d = inp("w1h", [128, KD * D], bf16)
    w1t_d = inp("w1t", [128, KD * D], bf16)
    f_d = inp("f_c", [128, NC_F], bf16)

    y = nc.dram_tensor("y", [2, NH], f32, kind="ExternalOutput")

    from contextlib import ExitStack
    with tile.TileContext(nc) as tc, ExitStack() as _ctx:
        sbw = _ctx.enter_context(tc.tile_pool(name="sbw", bufs=1))   # persistent
        sbt = _ctx.enter_context(tc.tile_pool(name="sbt", bufs=3))   # rotating temps
        pu_cm = tc.tile_pool(name="pu", bufs=3, space="PSUM")
        pu = pu_cm.__enter__()

        # ---------------- t0: gpsimd: warm tile, eidx, gather, memsets -----
        warm = sbw.tile([1, 512], bf16, tag="warm")
        nc.gpsimd.memset(warm[:], 0.0)
        t_eidx = sbw.tile([128, 8], i16, tag="eidx")
        nc.gpsimd.dma_start(t_eidx[:], eidx_d[:])
        # entities replicated on partition blocks 0:32 / 32:64 / 64:96 / 96:128
        ent_raw = sbw.tile([128, 1, D], f32, tag="entraw")
        nc.gpsimd.dma_gather(ent_raw[:], x_b[:], t_eidx[:],
                             num_idxs=128, num_idxs_reg=128, elem_size=D)
        ent = ent_raw[0:NE, 0, :]

        # padded intermediates (bf16) + border-only memsets
        img3 = sbw.tile([3, 34 * 34], bf16, tag="img3")
        c1p = sbw.tile([64, 34 * 34], bf16, tag="c1p")
        p1p = sbw.tile([64, 18 * 18], bf16, tag="p1p")
        c2p = sbw.tile([128, 18 * 18], bf16, tag="c2p")
        p2p = sbw.tile([128, 10 * 10], bf16, tag="p2p")
        u2p0 = sbw.tile([128, 18 * 18], bf16, tag="u2p0")
        u2p1 = sbw.tile([128, 18 * 18], bf16, tag="u2p1")
        att2p = sbw.tile([128, 18 * 18], bf16, tag="att2p")
        u1p = sbw.tile([128, 34 * 34], bf16, tag="u1p")
        att1p = sbw.tile([64, 34 * 34], bf16, tag="att1p")

        nc.gpsimd.memset(img3[:], 0.0)
        ones_bf = sbw.tile([1, 128], bf16, tag="ones")
        nc.gpsimd.memset(ones_bf[:], 1.0)

        def borders(t, n):
            v = t[:].rearrange("c (h w) -> c h w", h=n, w=n)
            nc.gpsimd.memset(v[:, 0:n:n - 1, :], 0.0)
            nc.gpsimd.memset(v[:, :, 0:n:n - 1], 0.0)

        for t, n in ((c1p, 34), (p1p, 18), (c2p, 18), (p2p, 10), (u2p0, 18),
                     (u2p1, 18), (att2p, 18), (u1p, 34), (att1p, 34)):
            borders(t, n)

        # ---------------- sync-engine DMA chunks (just-in-time order) ------
        def load(dram, shape, dt, tag, eng=None):
            t = sbw.tile(shape, dt, tag=tag)
            (eng or nc.sync).dma_start(t[:], dram[:])
            return t

        t_mf = load(mf_d, [128, NC_MF], f32, "mf")
        t_a64 = load(a64_d, [64, NC_A64], bf16, "a64")
        t_bott = load(bott_d, [128, NC_BOTT], bf16, "bott")
        t_ag2 = load(ag2_d, [128, NC_AG2], bf16, "ag2")
        t_pidx = load(pidx_d, [128, NH // 16], i16, "pidx")

        mov_h = sbw.tile([96, NH], bf16, tag="movh")
        mov_t = sbw.tile([96, NH], bf16, tag="movt")
        nc.sync.dma_start(mov_h[64:96, :], ohm_d[0:32, :])
        nc.sync.dma_start(mov_t[64:96, :], ohm_d[32:64, :])

        def vw(tile_, lay, name, shape=None):
            row0, r, c0, c = lay[name]
            ap = tile_[row0:row0 + r, c0:c0 + c]
            if shape is not None and len(shape) > 2:
                pat = {3: "p (a b) -> p a b", 4: "p (a b c) -> p a b c"}[len(shape)]
                kw = dict(zip("abc", shape[1:]))
                ap = ap.rearrange(pat, **kw)
            return ap

        t_ident = vw(t_a64, LAY_A64, "ident")
        t_enc1w = vw(t_a64, LAY_A64, "enc1w3", (3, 3, 64))
        t_enc2w = vw(t_a64, LAY_A64, "enc2w", (64, 9, 128))
        t_dec1wb = vw(t_a64, LAY_A64, "dec1wb", (64, 9, 64))
        t_ag1wx = vw(t_a64, LAY_A64, "ag1wx")
        t_ag1ps = vw(t_a64, LAY_A64, "ag1ps")
        t_bottw = vw(t_bott, LAY_BOTT, "bott", (128, 9, 256))
        t_ag2wg = vw(t_ag2, LAY_AG2, "ag2wg", (128, 2, 128))
        t_ag2wx = vw(t_ag2, LAY_AG2, "ag2wx")
        t_ag2ps = vw(t_ag2, LAY_AG2, "ag2ps")

        t_emask = vw(t_mf, LAY_MF, "emask")
        t_enc1b = vw(t_mf, LAY_MF, "enc1b")
        t_enc2b = vw(t_mf, LAY_MF, "enc2b")
        t_bottb = vw(t_mf, LAY_MF, "bottb")
        t_dec2bb = vw(t_mf, LAY_MF, "dec2b_b")
        t_dec1b = vw(t_mf, LAY_MF, "dec1b")
        t_hbp = vw(t_mf, LAY_MF, "hbp")
        t_tbp = vw(t_mf, LAY_MF, "tbp")
        t_decb = vw(t_mf, LAY_MF, "decb")

        # ---------------- PE warmup chain (p-state ramp) -------------------
        p_warm = pu.tile([1, 512], f32, tag="pu")
        NWARM = 5
        for i in range(NWARM):
            nc.tensor.matmul(p_warm[:], warm[0:1, 0:1], warm[:],
                             start=(i == 0), stop=(i == NWARM - 1))

        # ---------------- front-end: norms + transposes + cos --------------
        sq_scr = sbt.tile([128, D], bf16, tag="t")
        ss = sbw.tile([128, 1], f32, tag="ss")
        nc.scalar.activation(sq_scr[0:NE, :], ent, AF.Square, accum_out=ss[0:NE, :])
        # second copy of the norms on partitions 64:128 (scales for the
        # stacked extractor weights, which live at rows 64:96 / 96:128)
        nc.scalar.activation(sq_scr[64:96, :], ent_raw[64:96, 0, :], AF.Square,
                             accum_out=ss[64:96, :])
        normc = sbw.tile([128, 1], f32, tag="normc")
        nc.scalar.activation(normc[0:NE, :], ss[0:NE, :], AF.Sqrt)
        nc.scalar.activation(normc[64:96, :], ss[64:96, :], AF.Sqrt)
        # dummy sigmoid -> hoist the 2nd act-table load off the critical path
        dummy = sbt.tile([1, 1], f32, tag="dum")
        nc.scalar.activation(dummy[:], ss[0:1, :], AF.Sigmoid)
        nc.vector.tensor_single_scalar(normc[0:NE, :], normc[0:NE, :], 1e-13, op=OP.max)
        nc.vector.tensor_single_scalar(normc[64:96, :], normc[64:96, :], 1e-13, op=OP.max)
        rinv = sbw.tile([NE, 1], f32, tag="rinv")
        nc.vector.reciprocal(rinv[:], normc[0:NE, :])
        nc.vector.tensor_tensor(out=rinv[:], in0=rinv[:], in1=t_emask, op=OP.mult)
        nrm = sbw.tile([NE, D], bf16, tag="nrm")
        nc.vector.tensor_scalar(out=nrm[:], in0=ent, scalar1=rinv[:],
                                scalar2=None, op0=OP.mult)

        nrmT = sbw.tile([128, KD, NE], bf16, tag="nrmT")
        for k in range(KD):
            p_t = pu.tile([128, NE], bf16, tag="pu")
            nc.tensor.transpose(p_t[:], nrm[:, k * 128:(k + 1) * 128], t_ident)
            nc.vector.tensor_copy(nrmT[:, k, :], p_t[:])

        p_cos = pu.tile([NE, NE], f32, tag="pu")
        for k in range(KD):
            nc.tensor.matmul(p_cos[:], nrmT[:, k, :], nrmT[:, k, :],
                             start=(k == 0), stop=(k == KD - 1))
        s_cos = sbw.tile([NE, NE], bf16, tag="scos")
        nc.vector.tensor_copy(s_cos[:], p_cos[:])

        # ---------------- image staging: 3 column-shifted padded copies ----
        img3v = img3[:].rearrange("c (h w) -> c h w", h=34, w=34)
        nc.sync.dma_start(img3v[0:1, 1:33, 2:34], s_cos[:])
        nc.scalar.dma_start(img3v[1:2, 1:33, 1:33], s_cos[:])
        nc.gpsimd.dma_start(img3v[2:3, 1:33, 0:32], s_cos[:])

        # remaining weight chunks ride the sync queue AFTER the image DMA
        t_dec2wa = load(dec2a_d, [128, NC_DEC2A], bf16, "dec2a")
        t_dec2wa = t_dec2wa[:].rearrange("p (a b c) -> p a b c", a=2, b=9, c=128)
        t_dec2wb = load(dec2b_d, [128, NC_DEC2B], bf16, "dec2b")
        t_dec2wb = t_dec2wb[:].rearrange("p (b c) -> p b c", b=9, c=128)
        t_b128 = load(b128_d, [128, NC_B128], bf16, "b128")
        t_dec1wa = vw(t_b128, LAY_B128, "dec1wa", (128, 9, 64))
        t_ag1wg = vw(t_b128, LAY_B128, "ag1wg")
        t_w1h = load(w1h_d, [128, KD * D], bf16, "w1h")
        t_w1h = t_w1h[:].rearrange("p (k d) -> p k d", k=KD)
        t_w1t = load(w1t_d, [128, KD * D], bf16, "w1t")
        t_w1t = t_w1t[:].rearrange("p (k d) -> p k d", k=KD)
        t_f = load(f_d, [128, NC_F], bf16, "f")
        stk_h = vw(t_f, LAY_F, "stk_h", (128, KD, 128))
        stk_t = vw(t_f, LAY_F, "stk_t", (128, KD, 128))
        t_wdec = vw(t_f, LAY_F, "wdec", (128, G, 128))
        t_smat = vw(t_f, LAY_F, "smat")

        # ---------------- enc1: 2 halves x 3 dy matmuls (K=3) --------------
        c1pv = c1p[:].rearrange("c (h w) -> c h w", h=34, w=34)
        p_c1 = pu.tile([64, 1024], f32, tag="pu")
        for hh in range(2):
            for dy in range(3):
                rows = slice(dy + 16 * hh, dy + 16 * hh + 16)
                nc.tensor.matmul(p_c1[:, hh * 512:(hh + 1) * 512],
                                 t_enc1w[:, dy, :],
                                 img3v[0:3, rows, 1:33],
                                 start=(dy == 0), stop=(dy == 2))
        for hh in range(2):
            nc.scalar.activation(c1pv[:, 1 + 16 * hh:17 + 16 * hh, 1:33],
                                 p_c1[:, hh * 512:(hh + 1) * 512].rearrange(
                                     "c (h w) -> c h w", h=16, w=32),
                                 AF.Relu, bias=t_enc1b)

        # ---------------- pool1 (gpsimd) -----------------------------------
        p1pv = p1p[:].rearrange("c (h w) -> c h w", h=18, w=18)
        tmp = sbt.tile([64, 16, 16], bf16, tag="t")
        nc.gpsimd.tensor_max(tmp[:], c1pv[:, 1:33:2, 1:33:2], c1pv[:, 1:33:2, 2:34:2])
        nc.gpsimd.tensor_max(tmp[:], tmp[:], c1pv[:, 2:34:2, 1:33:2])
        nc.gpsimd.tensor_max(p1pv[:, 1:17, 1:17], tmp[:], c1pv[:, 2:34:2, 2:34:2])

        # ---------------- enc2: 9 taps K=64 --------------------------------
        p_c2 = pu.tile([128, 256], f32, tag="pu")
        for tap in range(9):
            dy, dx = tap // 3, tap % 3
            nc.tensor.matmul(p_c2[:], t_enc2w[:, tap, :],
                             p1pv[:, dy:dy + 16, dx:dx + 16],
                             start=(tap == 0), stop=(tap == 8))
        c2pv = c2p[:].rearrange("c (h w) -> c h w", h=18, w=18)
        nc.scalar.activation(c2pv[:, 1:17, 1:17],
                             p_c2[:].rearrange("c (h w) -> c h w", h=16, w=16),
                             AF.Relu, bias=t_enc2b)

        # ---------------- pool2 (gpsimd) -----------------------------------
        p2pv = p2p[:].rearrange("c (h w) -> c h w", h=10, w=10)
        tmp2 = sbt.tile([128, 8, 8], bf16, tag="t")
        nc.gpsimd.tensor_max(tmp2[:], c2pv[:, 1:17:2, 1:17:2], c2pv[:, 1:17:2, 2:18:2])
        nc.gpsimd.tensor_max(tmp2[:], tmp2[:], c2pv[:, 2:18:2, 1:17:2])
        nc.gpsimd.tensor_max(p2pv[:, 1:9, 1:9], tmp2[:], c2pv[:, 2:18:2, 2:18:2])

        # ---------------- bottleneck: 9 taps x 2 M-chunks, K=128 -----------
        c3 = []
        for mc in range(2):
            p_c3 = pu.tile([128, 64], f32, tag="pu")
            for tap in range(9):
                dy, dx = tap // 3, tap % 3
                nc.tensor.matmul(p_c3[:], t_bottw[:, tap, mc * 128:(mc + 1) * 128],
                                 p2pv[:, dy:dy + 8, dx:dx + 8],
                                 start=(tap == 0), stop=(tap == 8))
            c3s = sbt.tile([128, 8, 8], bf16, tag=f"c3_{mc}")
            nc.scalar.activation(c3s[:], p_c3[:].rearrange("c (h w) -> c h w", h=8, w=8),
                                 AF.Relu, bias=t_bottb[:, mc:mc + 1])
            c3.append(c3s)

        # ---------------- up2 ----------------------------------------------
        u2p0v = u2p0[:].rearrange("c (h w) -> c h w", h=18, w=18)
        u2p1v = u2p1[:].rearrange("c (h w) -> c h w", h=18, w=18)
        for src, dv in ((c3[0], u2p0v), (c3[1], u2p1v)):
            for i in range(2):
                for j in range(2):
                    nc.vector.tensor_copy(dv[:, 1 + i:17:2, 1 + j:17:2], src[:])

        # ---------------- attention gate 2 ---------------------------------
        p_a2 = pu.tile([128, 256], f32, tag="pu")
        nc.tensor.matmul(p_a2[:], t_ag2wg[:, 0, :], u2p0v[:, 1:17, 1:17],
                         start=True, stop=False)
        nc.tensor.matmul(p_a2[:], t_ag2wg[:, 1, :], u2p1v[:, 1:17, 1:17],
                         start=False, stop=False)
        nc.tensor.matmul(p_a2[:], t_ag2wx, c2pv[:, 1:17, 1:17],
                         start=False, stop=True)
        r2 = sbt.tile([128, 256], bf16, tag="t")
        nc.scalar.activation(r2[:], p_a2[:], AF.Relu)
        p_g2 = pu.tile([1, 256], f32, tag="pu")
        nc.tensor.matmul(p_g2[:], t_ag2ps, r2[:])
        a2 = sbt.tile([1, 256], bf16, tag="a2")
        nc.scalar.activation(a2[:], p_g2[:], AF.Sigmoid)
        p_a2b = pu.tile([128, 256], f32, tag="pu")
        nc.tensor.matmul(p_a2b[:], ones_bf[:], a2[:])
        att2pv = att2p[:].rearrange("c (h w) -> c h w", h=18, w=18)
        nc.vector.tensor_mul(att2pv[:, 1:17, 1:17],
                             p_a2b[:].rearrange("c (h w) -> c h w", h=16, w=16),
                             c2pv[:, 1:17, 1:17])

        # ---------------- dec2: 27 taps (u2 chunks first, att2 last) -------
        p_d2 = pu.tile([128, 256], f32, tag="pu")
        n_mm = 0
        for kc in range(2):
            src = (u2p0v, u2p1v)[kc]
            for tap in range(9):
                dy, dx = tap // 3, tap % 3
                nc.tensor.matmul(p_d2[:], t_dec2wa[:, kc, tap, :],
                                 src[:, dy:dy + 16, dx:dx + 16],
                                 start=(n_mm == 0), stop=False)
                n_mm += 1
        for tap in range(9):
            dy, dx = tap // 3, tap % 3
            nc.tensor.matmul(p_d2[:], t_dec2wb[:, tap, :],
                             att2pv[:, dy:dy + 16, dx:dx + 16],
                             start=False, stop=(tap == 8))
        d2s = sbt.tile([128, 256], bf16, tag="d2s")
        nc.scalar.activation(d2s[:], p_d2[:], AF.Relu, bias=t_dec2bb)

        # ---------------- up1 ----------------------------------------------
        u1pv = u1p[:].rearrange("c (h w) -> c h w", h=34, w=34)
        d2v = d2s[:].rearrange("c (h w) -> c h w", h=16, w=16)
        for i in range(2):
            for j in range(2):
                nc.vector.tensor_copy(u1pv[:, 1 + i:33:2, 1 + j:33:2], d2v[:])

        # ---------------- attention gate 1 ---------------------------------
        p_a1 = pu.tile([64, 1024], f32, tag="pu")
        for hh in range(2):
            rows = slice(1 + 16 * hh, 17 + 16 * hh)
            nc.tensor.matmul(p_a1[:, hh * 512:(hh + 1) * 512], t_ag1wx,
                             c1pv[:, rows, 1:33], start=True, stop=False)
            nc.tensor.matmul(p_a1[:, hh * 512:(hh + 1) * 512], t_ag1wg,
                             u1pv[:, rows, 1:33], start=False, stop=True)
        r1 = sbt.tile([64, 1024], bf16, tag="t")
        nc.scalar.activation(r1[:], p_a1[:], AF.Relu)
        p_g1 = pu.tile([1, 1024], f32, tag="pu")
        for hh in range(2):
            nc.tensor.matmul(p_g1[:, hh * 512:(hh + 1) * 512], t_ag1ps,
                             r1[:, hh * 512:(hh + 1) * 512])
        a1 = sbt.tile([1, 1024], bf16, tag="a1")
        nc.scalar.activation(a1[:], p_g1[:], AF.Sigmoid)
        p_a1b = pu.tile([64, 1024], f32, tag="pu")
        for hh in range(2):
            nc.tensor.matmul(p_a1b[:, hh * 512:(hh + 1) * 512], ones_bf[:, :64],
                             a1[:, hh * 512:(hh + 1) * 512])
        att1pv = att1p[:].rearrange("c (h w) -> c h w", h=34, w=34)
        nc.vector.tensor_mul(att1pv[:, 1:33, 1:33],
                             p_a1b[:].rearrange("c (h w) -> c h w", h=32, w=32),
                             c1pv[:, 1:33, 1:33])

        # ---------------- dec1 half 0 + EW-head + dec1 half 1 + EW-tail ----
        d1 = sbw.tile([64, 1024], f32, tag="d1")
        p_d1 = pu.tile([64, 1024], f32, tag="pu")
        for hh in range(2):
            cols = slice(hh * 512, (hh + 1) * 512)
            n_mm = 0
            for tap in range(9):
                dy, dx = tap // 3, tap % 3
                rows = slice(dy + 16 * hh, dy + 16 * hh + 16)
                nc.tensor.matmul(p_d1[:, cols], t_dec1wa[:, tap, :],
                                 u1pv[:, rows, dx:dx + 32],
                                 start=(n_mm == 0), stop=False)
                n_mm += 1
            for tap in range(9):
                dy, dx = tap // 3, tap % 3
                rows = slice(dy + 16 * hh, dy + 16 * hh + 16)
                nc.tensor.matmul(p_d1[:, cols], t_dec1wb[:, tap, :],
                                 att1pv[:, rows, dx:dx + 32],
                                 start=False, stop=(n_mm == 17))
                n_mm += 1
            if hh == 0:
                # EW-head premultiply: (ent @ head_w[:768]) at rows 64:96
                p_ewh = pu.tile([128, D], f32, tag="pu")
                for k in range(KD):
                    for n0, n1 in ((0, 512), (512, D)):
                        nc.tensor.matmul(p_ewh[64:96, n0:n1], nrmT[:, k, :],
                                         t_w1h[:, k, n0:n1],
                                         start=(k == 0), stop=(k == KD - 1))
                nc.scalar.activation(
                    stk_h[64:96, :, :].rearrange("p a b -> p (a b)"),
                    p_ewh[64:96, :], AF.Copy, scale=normc[64:96, :])
        nc.scalar.activation(d1[:], p_d1[:], AF.Relu, bias=t_dec1b)

        p_ewt = pu.tile([128, D], f32, tag="pu")
        for k in range(KD):
            for n0, n1 in ((0, 512), (512, D)):
                nc.tensor.matmul(p_ewt[64:96, n0:n1], nrmT[:, k, :],
                                 t_w1t[:, k, n0:n1],
                                 start=(k == 0), stop=(k == KD - 1))
        nc.scalar.activation(stk_t[64:96, :, :].rearrange("p a b -> p (a b)"),
                             p_ewt[64:96, :], AF.Copy, scale=normc[64:96, :])

        # ---------------- d1 gather -> mov rows 0:64 ------------------------
        d1g = sbt.tile([64, NH], f32, tag="d1g")
        nc.gpsimd.ap_gather(d1g[:].rearrange("c (n o) -> c n o", o=1),
                            d1[:].rearrange("c (n o) -> c n o", o=1), t_pidx[:],
                            channels=64, num_elems=1024, d=1, num_idxs=NH)
        nc.vector.tensor_copy(mov_h[0:64, :], d1g[:])
        nc.vector.tensor_copy(mov_t[0:64, :], d1g[:])

        pu_cm.__exit__(None, None, None)

        # ---------------- pair features + decoder --------------------------
        hsT = sbw.tile([128, KD, NH], bf16, tag="hsT")
        tsT = sbw.tile([128, KD, NH], bf16, tag="tsT")
        ph_cm = tc.tile_pool(name="ph", bufs=4, space="PSUM")
        ph = ph_cm.__enter__()
        pd_cm = tc.tile_pool(name="pd", bufs=2, space="PSUM")
        pd = pd_cm.__enter__()
        po_cm = tc.tile_pool(name="po", bufs=1, space="PSUM")
        po = po_cm.__enter__()
        p_out = po.tile([2, NH], f32, tag="po")
        for k in range(KD):
            for (stk, mv, bp, dstT) in ((stk_h, mov_h, t_hbp, hsT),
                                        (stk_t, mov_t, t_tbp, tsT)):
                p_hs = ph.tile([128, NH], f32, tag="ph")
                nc.tensor.matmul(p_hs[:], stk[0:96, k, :], mv[:])
                nc.scalar.activation(dstT[:, k, :], p_hs[:],
                                     AF.Tanh, bias=bp[:, k:k + 1])
            for half in range(2):
                g = 2 * k + half
                rows = slice(half * 64, (half + 1) * 64)
                p_u = pd.tile([128, NH], f32, tag="pd")
                nc.tensor.matmul(p_u[:], t_wdec[rows, g, :], tsT[rows, k, :])
                v = sbt.tile([128, NH], bf16, tag="v")
                eng = nc.vector if half == 0 else nc.gpsimd
                eng.tensor_mul(v[0:64, :], p_u[0:64, :], hsT[rows, k, :])
                eng.tensor_mul(v[64:128, :], p_u[64:128, :], hsT[rows, k, :])
                nc.tensor.matmul(p_out[:], t_smat, v[:],
                                 start=(g == 0), stop=(g == G - 1))
        out_sb = sbt.tile([2, NH], f32, tag="out")
        nc.scalar.activation(out_sb[:], p_out[:], AF.Identity, bias=t_decb)
        nc.sync.dma_start(y[:], out_sb[:])
        po_cm.__exit__(None, None, None)
        pd_cm.__exit__(None, None, None)
        ph_cm.__exit__(None, None, None)

    nc.compile()
    return nc


def _wrap16(idx, n_slots):
    """int16 index layout for gpsimd gathers: wrapped in 16 partitions,
    replicated across the 8 gpsimd cores."""
    out = np.zeros((128, n_slots), np.int16)
    for j, v in enumerate(idx):
        out[np.arange(8) * 16 + j % 16, j // 16] = v
    return out


def _bf(a):
    import ml_dtypes
    return np.asarray(a, np.float32).astype(ml_dtypes.bfloat16)


def _fill(lay, ncols, rows, dtype, vals):
    out = np.zeros((rows, ncols), dtype=dtype)
    for name, arr in vals.items():
        r0, r, c0, c = lay[name]
        a = np.asarray(arr)
        if a.ndim != 2:
            a = a.reshape(r, c)
        out[r0:r0 + a.shape[0], c0:c0 + a.shape[1]] = a
    return out


def pack_inputs(inputs):
    import ml_dtypes
    bfd = ml_dtypes.bfloat16
    x = np.asarray(inputs["x"], np.float32)
    entity_pos = np.asarray(inputs["entity_pos"])
    hts = np.asarray(inputs["hts"])

    def W(name):
        return np.asarray(inputs[name], np.float32)

    head_w, tail_w = W("head_w"), W("tail_w")
    fin_w = W("fin_w").reshape(OUT_CH, 64)
    fin_b = W("fin_b")
    w2h_f = fin_w.T @ head_w[D:]          # [64, 768]
    w2t_f = fin_w.T @ tail_w[D:]
    hb_f = W("head_b") + fin_b @ head_w[D:]
    tb_f = W("tail_b") + fin_b @ tail_w[D:]

    a64 = _fill(LAY_A64, NC_A64, 64, bfd, {
        "ident": _bf(np.eye(NE)),
        "enc1w3": _bf(W("enc1_w").reshape(64, 3, 3).transpose(2, 1, 0).reshape(3, 192)),
        "enc2w": _bf(W("enc2_w").reshape(128, 64, 9).transpose(1, 2, 0).reshape(64, 1152)),
        "dec1wb": _bf(W("dec1_w").reshape(64, 192, 9).transpose(1, 2, 0)[128:].reshape(64, 576)),
        "ag1wx": _bf(W("ag1_wx").reshape(64, 64).T),
        "ag1ps": _bf(W("ag1_psi").reshape(1, 64).T),
    })
    bott_c = _fill(LAY_BOTT, NC_BOTT, 128, bfd, {
        "bott": _bf(W("bott_w").reshape(256, 128, 9).transpose(1, 2, 0).reshape(128, 2304)),
    })
    ag2_c = _fill(LAY_AG2, NC_AG2, 128, bfd, {
        "ag2wg": _bf(W("ag2_wg").reshape(128, 256).T.reshape(2, 128, 128)
                     .transpose(1, 0, 2).reshape(128, 256)),
        "ag2wx": _bf(W("ag2_wx").reshape(128, 128).T),
        "ag2ps": _bf(W("ag2_psi").reshape(1, 128).T),
    })
    d2w = W("dec2_w").reshape(128, 384, 9).transpose(1, 2, 0)\
        .reshape(3, 128, 9, 128).transpose(1, 0, 2, 3)       # [128, kc, 9, 128]
    dec2a_c = _fill(LAY_DEC2A, NC_DEC2A, 128, bfd,
                    {"dec2a": _bf(d2w[:, 0:2].reshape(128, 2304))})
    dec2b_c = _fill(LAY_DEC2B, NC_DEC2B, 128, bfd,
                    {"dec2b": _bf(d2w[:, 2].reshape(128, 1152))})
    b128 = _fill(LAY_B128, NC_B128, 128, bfd, {
        "dec1wa": _bf(W("dec1_w").reshape(64, 192, 9).transpose(1, 2, 0)[:128]
                      .reshape(128, 576)),
        "ag1wg": _bf(W("ag1_wg").reshape(64, 128).T),
    })
    w1h = _bf(head_w[:D].reshape(KD, 128, D).transpose(1, 0, 2).reshape(128, KD * D))
    w1t = _bf(tail_w[:D].reshape(KD, 128, D).transpose(1, 0, 2).reshape(128, KD * D))

    stk_h = np.zeros((128, KD * 128), np.float32)
    stk_h[0:64] = w2h_f.reshape(64, KD, 128).reshape(64, KD * 128)
    stk_t = np.zeros((128, KD * 128), np.float32)
    stk_t[0:64] = w2t_f.reshape(64, KD, 128).reshape(64, KD * 128)
    wd = W("decoder_w").reshape(G, 64, 64, 2).transpose(2, 0, 3, 1).reshape(64, G * 128)
    smat = np.zeros((128, 2), np.float32)
    smat[:64, 0] = 1.0
    smat[64:, 1] = 1.0
    f_c = _fill(LAY_F, NC_F, 128, bfd, {
        "stk_h": _bf(stk_h),
        "stk_t": _bf(stk_t),
        "wdec": _bf(np.concatenate([wd, wd], axis=0)),
        "smat": _bf(smat),
    })

    mf = _fill(LAY_MF, NC_MF, 128, np.float32, {
        "enc1b": W("enc1_b").reshape(64, 1),
        "enc2b": W("enc2_b").reshape(128, 1),
        "bottb": W("bott_b").reshape(2, 128).T,
        "dec2b_b": W("dec2_b").reshape(128, 1),
        "dec1b": W("dec1_b").reshape(64, 1),
        "hbp": hb_f.reshape(KD, 128).T,
        "tbp": tb_f.reshape(KD, 128).T,
        "decb": W("decoder_b").reshape(2, 1),
        # emask filled per-core below
    })

    shared = dict(a64=a64, bott_c=bott_c, ag2_c=ag2_c, dec2a_c=dec2a_c,
                  dec2b_c=dec2b_c, b128=b128, w1h=w1h, w1t=w1t, f_c=f_c)

    in_maps = []
    for c in range(NCORES):
        b, h = c // 2, c % 2
        m = dict(shared)
        m["x_b"] = np.ascontiguousarray(x[b])
        start = entity_pos[b, :, 0].astype(np.int64)
        idx = np.minimum(start + 1, L - 1).astype(np.int16)
        m["eidx"] = _wrap16(np.tile(idx, 4), 8)
        mfc = mf.copy()
        r0, r, c0, cc = LAY_MF["emask"]
        mfc[r0:r0 + NE, c0] = (start + 1 < L).astype(np.float32)
        m["mf"] = mfc
        hi = hts[b, h * NH:(h + 1) * NH, 0].astype(np.int64)
        ti = hts[b, h * NH:(h + 1) * NH, 1].astype(np.int64)
        ohm = np.zeros((64, NH), np.float32)
        ohm[hi, np.arange(NH)] = 1.0
        ohm[32 + ti, np.arange(NH)] = 1.0
        m["ohm"] = ohm.astype(bfd)
        m["pidx"] = _wrap16((hi * NE + ti).astype(np.int16), NH // 16)
        in_maps.append(m)
    return in_maps


_NC_CACHE = None


def get_nc():
    global _NC_CACHE
    if _NC_CACHE is None:
        _NC_CACHE = build_nc()
    return _NC_CACHE


def kernel(**inputs):
    nc = get_nc()
    in_maps = pack_inputs(inputs)
    res = run_bass_kernel_spmd(nc, in_maps, core_ids=list(range(NCORES)))
    out = np.empty((B * P, 2), np.float32)
    for c in range(NCORES):
        b, h = c // 2, c % 2
        yc = res.results[c]["y"]                  # [2, NH]
        out[b * P + h * NH:b * P + (h + 1) * NH, :] = yc.T
    return out


# revision 7
# speedup vs baseline: 1.3041x; 1.0336x over previous
"""Trainium2 Bass kernel for nn_CoreferenceResolver (coref UNet + pair decoder).

Sharding: core c handles batch b=c//2 and pair-half h=c%2 (496 of 992 pairs).
The gather/cosine/UNet stages are replicated on the two cores sharing a batch;
the extractor linears and group-bilinear decoder are sharded over pairs.

v1 design notes (vs the f32r baseline):
- all weights/activations bf16 (halves DMA bytes, PE still 1 cycle/row)
- fin 1x1 conv + amap gathers + W2 matmuls folded into host-precomputed
  W2' = fin_w^T @ head_w[768:] and a single d1 gather
- extractor = one stacked K=128 matmul per (k, extractor):
  mov rows 0:64 d1[pairs], 64:96 one-hot(hi) (host), 96:128 one-hot(ti)
- enc1 via 3 column-shifted padded images (K=3 matmuls, 6 total)
- weights arrive as a few packed DMA chunks ordered just-in-time so the
  cos->image DMA never queues behind megabytes of weight traffic
- PE warmup chain holds the p-state ramp so real matmuls price at full speed
"""
import os
import sys

for _p in ("/opt/trn_rl_repo",):
    if os.path.isdir(_p) and _p not in sys.path:
        sys.path.insert(0, _p)

import numpy as np

import concourse.bass as bass
import concourse.tile as tile
from concourse import bacc, mybir
from concourse.bass_utils import run_bass_kernel_spmd

f32 = mybir.dt.float32
i16 = mybir.dt.int16
bf16 = mybir.dt.bfloat16
AF = mybir.ActivationFunctionType
OP = mybir.AluOpType

B, L, D, H = 4, 1024, 768, 12
NE, P = 32, 992
BLOCK = 64
G = D // BLOCK          # 12 groups
OUT_CH = 256
NCORES = 8
NH = P // 2             # 496 pairs per core
KD = D // 128           # 6 chunks of the D dim

# ---------------------------------------------------------------------------
# packed-chunk layouts (shapes only; shared by build_nc and pack_inputs)
# entries: name -> (row0, rows, col0, cols); each chunk = one DRAM tensor.
# ---------------------------------------------------------------------------


def _mklayout(rows, entries):
    lay, col = {}, 0
    for name, r0, r, c in entries:
        lay[name] = (r0, r, col, c)
        col += c
    return lay, col


LAY_A64, NC_A64 = _mklayout(64, [
    ("ident", 0, 32, 32),
    ("enc1w3", 0, 3, 192),          # [dx, dy*64+c]
    ("enc2w", 0, 64, 1152),         # [64, 9*128]
    ("dec1wb", 0, 64, 576),         # [64, 9*64]
    ("ag1wx", 0, 64, 64),
    ("ag1ps", 0, 64, 1),
])
LAY_BOTT, NC_BOTT = _mklayout(128, [("bott", 0, 128, 2304)])   # [128, 9*256]
LAY_AG2, NC_AG2 = _mklayout(128, [
    ("ag2wg", 0, 128, 256),         # [128, 2*128]
    ("ag2wx", 0, 128, 128),
    ("ag2ps", 0, 128, 1),
])
LAY_DEC2A, NC_DEC2A = _mklayout(128, [("dec2a", 0, 128, 2304)])  # kc 0:2
LAY_DEC2B, NC_DEC2B = _mklayout(128, [("dec2b", 0, 128, 1152)])  # kc 2
LAY_B128, NC_B128 = _mklayout(128, [
    ("dec1wa", 0, 128, 576),        # [128, 9*64]
    ("ag1wg", 0, 128, 64),
])
LAY_F, NC_F = _mklayout(128, [
    ("stk_h", 0, 128, 768),         # rows 0:64 W2h'; 64:96 <- EW1 (device)
    ("stk_t", 0, 128, 768),         # rows 0:64 W2t'; 64:96 <- EW1t (device)
    ("wdec", 0, 128, 1536),         # [128, 12*128] (rows duplicated)
    ("smat", 0, 128, 2),
])
LAY_MF, NC_MF = _mklayout(128, [
    ("emask", 0, 32, 1),
    ("enc1b", 0, 64, 1),
    ("enc2b", 0, 128, 1),
    ("bottb", 0, 128, 2),
    ("dec2b_b", 0, 128, 1),
    ("dec1b", 0, 64, 1),
    ("hbp", 0, 128, 6),
    ("tbp", 0, 128, 6),
    ("decb", 0, 2, 1),
])


def build_nc():
    nc = bacc.Bacc("TRN2", target_bir_lowering=False, debug=False, num_devices=NCORES)

    def inp(name, shape, dt=f32):
        return nc.dram_tensor(name, shape, dt, kind="ExternalInput")

    x_b = inp("x_b", [L, D])
    eidx_d = inp("eidx", [128, 8], i16)
    pidx_d = inp("pidx", [128, NH // 16], i16)
    ohm_d = inp("ohm", [64, NH], bf16)
    mf_d = inp("mf", [128, NC_MF])
    a64_d = inp("a64", [64, NC_A64], bf16)
    bott_d = inp("bott_c", [128, NC_BOTT], bf16)
    ag2_d = inp("ag2_c", [128, NC_AG2], bf16)
    dec2a_d = inp("dec2a_c", [128, NC_DEC2A], bf16)
    dec2b_d = inp("dec2b_c", [128, NC_DEC2B], bf16)
    b128_d = inp("b128", [128, NC_B128], bf16)
    w1h_d = inp("w1h", [128, KD * D], bf16)
    w1t_d = inp("w1t", [128, KD * D], bf16)
    f_d = inp("f_c", [128, NC_F], bf16)

    y = nc.dram_tensor("y", [2, NH], f32, kind="ExternalOutput")

    from contextlib import ExitStack
    with tile.TileContext(nc) as tc, ExitStack() as _ctx:
        sbw = _ctx.enter_context(tc.tile_pool(name="sbw", bufs=1))   # persistent
        sbt = _ctx.enter_context(tc.tile_pool(name="sbt", bufs=3))   # rotating temps
        pu_cm = tc.tile_pool(name="pu", bufs=3, space="PSUM")
        pu = pu_cm.__enter__()

        # ---------------- t0: gpsimd: warm tile, eidx, gather, memsets -----
        warm = sbw.tile([1, 512], bf16, tag="warm")
        nc.gpsimd.memset(warm[:], 0.0)
        t_eidx = sbw.tile([128, 8], i16, tag="eidx")
        nc.gpsimd.dma_start(t_eidx[:], eidx_d[:])
        # entities replicated on partition blocks 0:32 / 32:64 / 64:96 / 96:128
        ent_raw = sbw.tile([128, 1, D], f32, tag="entraw")
        nc.gpsimd.dma_gather(ent_raw[:], x_b[:], t_eidx[:],
                             num_idxs=128, num_idxs_reg=128, elem_size=D)
        ent = ent_raw[0:NE, 0, :]

        # padded intermediates (bf16) + border-only memsets
        img3 = sbw.tile([3, 34 * 34], bf16, tag="img3")
        c1p = sbw.tile([64, 34 * 34], bf16, tag="c1p")
        p1p = sbw.tile([64, 18 * 18], bf16, tag="p1p")
        c2p = sbw.tile([128, 18 * 18], bf16, tag="c2p")
        p2p = sbw.tile([128, 10 * 10], bf16, tag="p2p")
        u2p0 = sbw.tile([128, 18 * 18], bf16, tag="u2p0")
        u2p1 = sbw.tile([128, 18 * 18], bf16, tag="u2p1")
        att2p = sbw.tile([128, 18 * 18], bf16, tag="att2p")
        u1p = sbw.tile([128, 34 * 34], bf16, tag="u1p")
        att1p = sbw.tile([64, 34 * 34], bf16, tag="att1p")

        nc.gpsimd.memset(img3[:], 0.0)
        ones_bf = sbw.tile([1, 128], bf16, tag="ones")
        nc.gpsimd.memset(ones_bf[:], 1.0)

        def borders(t, n):
            v = t[:].rearrange("c (h w) -> c h w", h=n, w=n)
            nc.gpsimd.memset(v[:, 0:n:n - 1, :], 0.0)
            nc.gpsimd.memset(v[:, :, 0:n:n - 1], 0.0)

        for t, n in ((c1p, 34), (p1p, 18), (c2p, 18), (p2p, 10), (u2p0, 18),
                     (u2p1, 18), (att2p, 18), (u1p, 34), (att1p, 34)):
            borders(t, n)

        # ---------------- sync-engine DMA chunks (just-in-time order) ------
        def load(dram, shape, dt, tag, eng=None):
            t = sbw.tile(shape, dt, tag=tag)
            (eng or nc.sync).dma_start(t[:], dram[:])
            return t

        t_mf = load(mf_d, [128, NC_MF], f32, "mf")
        t_a64 = load(a64_d, [64, NC_A64], bf16, "a64")
        t_bott = load(bott_d, [128, NC_BOTT], bf16, "bott")
        t_ag2 = load(ag2_d, [128, NC_AG2], bf16, "ag2")
        t_pidx = load(pidx_d, [128, NH // 16], i16, "pidx")

        mov_h = sbw.tile([96, NH], bf16, tag="movh")
        mov_t = sbw.tile([96, NH], bf16, tag="movt")
        nc.sync.dma_start(mov_h[64:96, :], ohm_d[0:32, :])
        nc.sync.dma_start(mov_t[64:96, :], ohm_d[32:64, :])

        def vw(tile_, lay, name, shape=None):
            row0, r, c0, c = lay[name]
            ap = tile_[row0:row0 + r, c0:c0 + c]
            if shape is not None and len(shape) > 2:
                pat = {3: "p (a b) -> p a b", 4: "p (a b c) -> p a b c"}[len(shape)]
                kw = dict(zip("abc", shape[1:]))
                ap = ap.rearrange(pat, **kw)
            return ap

        t_ident = vw(t_a64, LAY_A64, "ident")
        t_enc1w = vw(t_a64, LAY_A64, "enc1w3", (3, 3, 64))
        t_enc2w = vw(t_a64, LAY_A64, "enc2w", (64, 9, 128))
        t_dec1wb = vw(t_a64, LAY_A64, "dec1wb", (64, 9, 64))
        t_ag1wx = vw(t_a64, LAY_A64, "ag1wx")
        t_ag1ps = vw(t_a64, LAY_A64, "ag1ps")
        t_bottw = vw(t_bott, LAY_BOTT, "bott", (128, 9, 256))
        t_ag2wg = vw(t_ag2, LAY_AG2, "ag2wg", (128, 2, 128))
        t_ag2wx = vw(t_ag2, LAY_AG2, "ag2wx")
        t_ag2ps = vw(t_ag2, LAY_AG2, "ag2ps")

        t_emask = vw(t_mf, LAY_MF, "emask")
        t_enc1b = vw(t_mf, LAY_MF, "enc1b")
        t_enc2b = vw(t_mf, LAY_MF, "enc2b")
        t_bottb = vw(t_mf, LAY_MF, "bottb")
        t_dec2bb = vw(t_mf, LAY_MF, "dec2b_b")
        t_dec1b = vw(t_mf, LAY_MF, "dec1b")
        t_hbp = vw(t_mf, LAY_MF, "hbp")
        t_tbp = vw(t_mf, LAY_MF, "tbp")
        t_decb = vw(t_mf, LAY_MF, "decb")

        # ---------------- PE warmup chain (p-state ramp) -------------------
        p_warm = pu.tile([1, 512], f32, tag="pu")
        NWARM = 5
        for i in range(NWARM):
            nc.tensor.matmul(p_warm[:], warm[0:1, 0:1], warm[:],
                             start=(i == 0), stop=(i == NWARM - 1))

        # ---------------- front-end: norms + transposes + cos --------------
        sq_scr = sbt.tile([128, D], bf16, tag="t")
        ss = sbw.tile([128, 1], f32, tag="ss")
        nc.scalar.activation(sq_scr[0:NE, :], ent, AF.Square, accum_out=ss[0:NE, :])
        # second copy of the norms on partitions 64:128 (scales for the
        # stacked extractor weights, which live at rows 64:96 / 96:128)
        nc.scalar.activation(sq_scr[64:96, :], ent_raw[64:96, 0, :], AF.Square,
                             accum_out=ss[64:96, :])
        normc = sbw.tile([128, 1], f32, tag="normc")
        nc.scalar.activation(normc[0:NE, :], ss[0:NE, :], AF.Sqrt)
        nc.scalar.activation(normc[64:96, :], ss[64:96, :], AF.Sqrt)
        # dummy sigmoid -> hoist the 2nd act-table load off the critical path
        dummy = sbt.tile([1, 1], f32, tag="dum")
        nc.scalar.activation(dummy[:], ss[0:1, :], AF.Sigmoid)
        nc.vector.tensor_single_scalar(normc[0:NE, :], normc[0:NE, :], 1e-13, op=OP.max)
        nc.vector.tensor_single_scalar(normc[64:96, :], normc[64:96, :], 1e-13, op=OP.max)
        rinv = sbw.tile([NE, 1], f32, tag="rinv")
        nc.vector.reciprocal(rinv[:], normc[0:NE, :])
        nc.vector.tensor_tensor(out=rinv[:], in0=rinv[:], in1=t_emask, op=OP.mult)
        nrm = sbw.tile([NE, D], bf16, tag="nrm")
        nc.vector.tensor_scalar(out=nrm[:], in0=ent, scalar1=rinv[:],
                                scalar2=None, op0=OP.mult)

        nrmT = sbw.tile([128, KD, NE], bf16, tag="nrmT")
        for k in range(KD):
            p_t = pu.tile([128, NE], bf16, tag="pu")
            nc.tensor.transpose(p_t[:], nrm[:, k * 128:(k + 1) * 128], t_ident)
            nc.vector.tensor_copy(nrmT[:, k, :], p_t[:])

        p_cos = pu.tile([NE, NE], f32, tag="pu")
        for k in range(KD):
            nc.tensor.matmul(p_cos[:], nrmT[:, k, :], nrmT[:, k, :],
                             start=(k == 0), stop=(k == KD - 1))
        s_cos = sbw.tile([NE, NE], bf16, tag="scos")
        nc.vector.tensor_copy(s_cos[:], p_cos[:])

        # ---------------- image staging: 3 column-shifted padded copies ----
        img3v = img3[:].rearrange("c (h w) -> c h w", h=34, w=34)
        nc.sync.dma_start(img3v[0:1, 1:33, 2:34], s_cos[:])
        nc.scalar.dma_start(img3v[1:2, 1:33, 1:33], s_cos[:])
        nc.gpsimd.dma_start(img3v[2:3, 1:33, 0:32], s_cos[:])

        # remaining weight chunks ride the sync queue AFTER the image DMA
        t_dec2wa = load(dec2a_d, [128, NC_DEC2A], bf16, "dec2a")
        t_dec2wa = t_dec2wa[:].rearrange("p (a b c) -> p a b c", a=2, b=9, c=128)
        t_dec2wb = load(dec2b_d, [128, NC_DEC2B], bf16, "dec2b")
        t_dec2wb = t_dec2wb[:].rearrange("p (b c) -> p b c", b=9, c=128)
        t_b128 = load(b128_d, [128, NC_B128], bf16, "b128")
        t_dec1wa = vw(t_b128, LAY_B128, "dec1wa", (128, 9, 64))
        t_ag1wg = vw(t_b128, LAY_B128, "ag1wg")
        t_w1h = load(w1h_d, [128, KD * D], bf16, "w1h")
        t_w1h = t_w1h[:].rearrange("p (k d) -> p k d", k=KD)
        t_w1t = load(w1t_d, [128, KD * D], bf16, "w1t")
        t_w1t = t_w1t[:].rearrange("p (k d) -> p k d", k=KD)
        t_f = load(f_d, [128, NC_F], bf16, "f")
        stk_h = vw(t_f, LAY_F, "stk_h", (128, KD, 128))
        stk_t = vw(t_f, LAY_F, "stk_t", (128, KD, 128))
        t_wdec = vw(t_f, LAY_F, "wdec", (128, G, 128))
        t_smat = vw(t_f, LAY_F, "smat")

        # ---------------- enc1: 2 halves x 3 dy matmuls (K=3) --------------
        c1pv = c1p[:].rearrange("c (h w) -> c h w", h=34, w=34)
        p_c1 = pu.tile([64, 1024], f32, tag="pu")
        for hh in range(2):
            for dy in range(3):
                rows = slice(dy + 16 * hh, dy + 16 * hh + 16)
                nc.tensor.matmul(p_c1[:, hh * 512:(hh + 1) * 512],
                                 t_enc1w[:, dy, :],
                                 img3v[0:3, rows, 1:33],
                                 start=(dy == 0), stop=(dy == 2))
        for hh in range(2):
            nc.scalar.activation(c1pv[:, 1 + 16 * hh:17 + 16 * hh, 1:33],
                                 p_c1[:, hh * 512:(hh + 1) * 512].rearrange(
                                     "c (h w) -> c h w", h=16, w=32),
                                 AF.Relu, bias=t_enc1b)

        # ---------------- pool1 (gpsimd) -----------------------------------
        p1pv = p1p[:].rearrange("c (h w) -> c h w", h=18, w=18)
        tmp = sbt.tile([64, 16, 16], bf16, tag="t")
        nc.vector.tensor_max(tmp[:], c1pv[:, 1:33:2, 1:33:2], c1pv[:, 1:33:2, 2:34:2])
        nc.vector.tensor_max(tmp[:], tmp[:], c1pv[:, 2:34:2, 1:33:2])
        nc.vector.tensor_max(p1pv[:, 1:17, 1:17], tmp[:], c1pv[:, 2:34:2, 2:34:2])

        # ---------------- enc2: 9 taps K=64 --------------------------------
        p_c2 = pu.tile([128, 256], f32, tag="pu")
        for tap in range(9):
            dy, dx = tap // 3, tap % 3
            nc.tensor.matmul(p_c2[:], t_enc2w[:, tap, :],
                             p1pv[:, dy:dy + 16, dx:dx + 16],
                             start=(tap == 0), stop=(tap == 8))
        c2pv = c2p[:].rearrange("c (h w) -> c h w", h=18, w=18)
        nc.scalar.activation(c2pv[:, 1:17, 1:17],
                             p_c2[:].rearrange("c (h w) -> c h w", h=16, w=16),
                             AF.Relu, bias=t_enc2b)

        # ---------------- pool2 (gpsimd) -----------------------------------
        p2pv = p2p[:].rearrange("c (h w) -> c h w", h=10, w=10)
        tmp2 = sbt.tile([128, 8, 8], bf16, tag="t")
        nc.vector.tensor_max(tmp2[:], c2pv[:, 1:17:2, 1:17:2], c2pv[:, 1:17:2, 2:18:2])
        nc.vector.tensor_max(tmp2[:], tmp2[:], c2pv[:, 2:18:2, 1:17:2])
        nc.vector.tensor_max(p2pv[:, 1:9, 1:9], tmp2[:], c2pv[:, 2:18:2, 2:18:2])

        # ---------------- bottleneck: 9 taps x 2 M-chunks, K=128 -----------
        c3 = []
        for mc in range(2):
            p_c3 = pu.tile([128, 64], f32, tag="pu")
            for tap in range(9):
                dy, dx = tap // 3, tap % 3
                nc.tensor.matmul(p_c3[:], t_bottw[:, tap, mc * 128:(mc + 1) * 128],
                                 p2pv[:, dy:dy + 8, dx:dx + 8],
                                 start=(tap == 0), stop=(tap == 8))
            c3s = sbt.tile([128, 8, 8], bf16, tag=f"c3_{mc}")
            nc.scalar.activation(c3s[:], p_c3[:].rearrange("c (h w) -> c h w", h=8, w=8),
                                 AF.Relu, bias=t_bottb[:, mc:mc + 1])
            c3.append(c3s)

        # ---------------- up2 ----------------------------------------------
        u2p0v = u2p0[:].rearrange("c (h w) -> c h w", h=18, w=18)
        u2p1v = u2p1[:].rearrange("c (h w) -> c h w", h=18, w=18)
        for src, dv in ((c3[0], u2p0v), (c3[1], u2p1v)):
            for i in range(2):
                for j in range(2):
                    nc.vector.tensor_copy(dv[:, 1 + i:17:2, 1 + j:17:2], src[:])

        # ---------------- attention gate 2 ---------------------------------
        p_a2 = pu.tile([128, 256], f32, tag="pu")
        nc.tensor.matmul(p_a2[:], t_ag2wg[:, 0, :], u2p0v[:, 1:17, 1:17],
                         start=True, stop=False)
        nc.tensor.matmul(p_a2[:], t_ag2wg[:, 1, :], u2p1v[:, 1:17, 1:17],
                         start=False, stop=False)
        nc.tensor.matmul(p_a2[:], t_ag2wx, c2pv[:, 1:17, 1:17],
                         start=False, stop=True)
        r2 = sbt.tile([128, 256], bf16, tag="t")
        nc.scalar.activation(r2[:], p_a2[:], AF.Relu)
        p_g2 = pu.tile([1, 256], f32, tag="pu")
        nc.tensor.matmul(p_g2[:], t_ag2ps, r2[:])
        a2 = sbt.tile([1, 256], bf16, tag="a2")
        nc.scalar.activation(a2[:], p_g2[:], AF.Sigmoid)
        p_a2b = pu.tile([128, 256], f32, tag="pu")
        nc.tensor.matmul(p_a2b[:], ones_bf[:], a2[:])
        att2pv = att2p[:].rearrange("c (h w) -> c h w", h=18, w=18)
        nc.vector.tensor_mul(att2pv[:, 1:17, 1:17],
                             p_a2b[:].rearrange("c (h w) -> c h w", h=16, w=16),
                             c2pv[:, 1:17, 1:17])

        # ---------------- dec2: 27 taps (u2 chunks first, att2 last) -------
        p_d2 = pu.tile([128, 256], f32, tag="pu")
        n_mm = 0
        for kc in range(2):
            src = (u2p0v, u2p1v)[kc]
            for tap in range(9):
                dy, dx = tap // 3, tap % 3
                nc.tensor.matmul(p_d2[:], t_dec2wa[:, kc, tap, :],
                                 src[:, dy:dy + 16, dx:dx + 16],
                                 start=(n_mm == 0), stop=False)
                n_mm += 1
        for tap in range(9):
            dy, dx = tap // 3, tap % 3
            nc.tensor.matmul(p_d2[:], t_dec2wb[:, tap, :],
                             att2pv[:, dy:dy + 16, dx:dx + 16],
                             start=False, stop=(tap == 8))
        d2s = sbt.tile([128, 256], bf16, tag="d2s")
        nc.scalar.activation(d2s[:], p_d2[:], AF.Relu, bias=t_dec2bb)

        # ---------------- up1 ----------------------------------------------
        u1pv = u1p[:].rearrange("c (h w) -> c h w", h=34, w=34)
        d2v = d2s[:].rearrange("c (h w) -> c h w", h=16, w=16)
        for i in range(2):
            for j in range(2):
                nc.vector.tensor_copy(u1pv[:, 1 + i:33:2, 1 + j:33:2], d2v[:])

        # ---------------- attention gate 1 ---------------------------------
        p_a1 = pu.tile([64, 1024], f32, tag="pu")
        for hh in range(2):
            rows = slice(1 + 16 * hh, 17 + 16 * hh)
            nc.tensor.matmul(p_a1[:, hh * 512:(hh + 1) * 512], t_ag1wx,
                             c1pv[:, rows, 1:33], start=True, stop=False)
            nc.tensor.matmul(p_a1[:, hh * 512:(hh + 1) * 512], t_ag1wg,
                             u1pv[:, rows, 1:33], start=False, stop=True)
        r1 = sbt.tile([64, 1024], bf16, tag="t")
        nc.scalar.activation(r1[:], p_a1[:], AF.Relu)
        p_g1 = pu.tile([1, 1024], f32, tag="pu")
        for hh in range(2):
            nc.tensor.matmul(p_g1[:, hh * 512:(hh + 1) * 512], t_ag1ps,
                             r1[:, hh * 512:(hh + 1) * 512])
        a1 = sbt.tile([1, 1024], bf16, tag="a1")
        nc.scalar.activation(a1[:], p_g1[:], AF.Sigmoid)
        p_a1b = pu.tile([64, 1024], f32, tag="pu")
        for hh in range(2):
            nc.tensor.matmul(p_a1b[:, hh * 512:(hh + 1) * 512], ones_bf[:, :64],
                             a1[:, hh * 512:(hh + 1) * 512])
        att1pv = att1p[:].rearrange("c (h w) -> c h w", h=34, w=34)
        nc.vector.tensor_mul(att1pv[:, 1:33, 1:33],
                             p_a1b[:].rearrange("c (h w) -> c h w", h=32, w=32),
                             c1pv[:, 1:33, 1:33])

        # ---------------- dec1 half 0 + EW-head + dec1 half 1 + EW-tail ----
        d1 = sbw.tile([64, 1024], f32, tag="d1")
        p_d1 = pu.tile([64, 1024], f32, tag="pu")
        for hh in range(2):
            cols = slice(hh * 512, (hh + 1) * 512)
            n_mm = 0
            for tap in range(9):
                dy, dx = tap // 3, tap % 3
                rows = slice(dy + 16 * hh, dy + 16 * hh + 16)
                nc.tensor.matmul(p_d1[:, cols], t_dec1wa[:, tap, :],
                                 u1pv[:, rows, dx:dx + 32],
                                 start=(n_mm == 0), stop=False)
                n_mm += 1
            for tap in range(9):
                dy, dx = tap // 3, tap % 3
                rows = slice(dy + 16 * hh, dy + 16 * hh + 16)
                nc.tensor.matmul(p_d1[:, cols], t_dec1wb[:, tap, :],
                                 att1pv[:, rows, dx:dx + 32],
                                 start=False, stop=(n_mm == 17))
                n_mm += 1
            if hh == 0:
                # EW-head premultiply: (ent @ head_w[:768]) at rows 64:96
                p_ewh = pu.tile([128, D], f32, tag="pu")
                for k in range(KD):
                    for n0, n1 in ((0, 512), (512, D)):
                        nc.tensor.matmul(p_ewh[64:96, n0:n1], nrmT[:, k, :],
                                         t_w1h[:, k, n0:n1],
                                         start=(k == 0), stop=(k == KD - 1))
                nc.scalar.activation(
                    stk_h[64:96, :, :].rearrange("p a b -> p (a b)"),
                    p_ewh[64:96, :], AF.Copy, scale=normc[64:96, :])
        nc.scalar.activation(d1[:], p_d1[:], AF.Relu, bias=t_dec1b)

        p_ewt = pu.tile([128, D], f32, tag="pu")
        for k in range(KD):
            for n0, n1 in ((0, 512), (512, D)):
                nc.tensor.matmul(p_ewt[64:96, n0:n1], nrmT[:, k, :],
                                 t_w1t[:, k, n0:n1],
                                 start=(k == 0), stop=(k == KD - 1))
        nc.scalar.activation(stk_t[64:96, :, :].rearrange("p a b -> p (a b)"),
                             p_ewt[64:96, :], AF.Copy, scale=normc[64:96, :])

        # ---------------- d1 gather -> mov rows 0:64 ------------------------
        d1g = sbt.tile([64, NH], f32, tag="d1g")
        nc.gpsimd.ap_gather(d1g[:].rearrange("c (n o) -> c n o", o=1),
                            d1[:].rearrange("c (n o) -> c n o", o=1), t_pidx[:],
                            channels=64, num_elems=1024, d=1, num_idxs=NH)
        nc.vector.tensor_copy(mov_h[0:64, :], d1g[:])
        nc.vector.tensor_copy(mov_t[0:64, :], d1g[:])

        pu_cm.__exit__(None, None, None)

        # ---------------- pair features + decoder --------------------------
        hsT = sbw.tile([128, KD, NH], bf16, tag="hsT")
        tsT = sbw.tile([128, KD, NH], bf16, tag="tsT")
        ph_cm = tc.tile_pool(name="ph", bufs=4, space="PSUM")
        ph = ph_cm.__enter__()
        pd_cm = tc.tile_pool(name="pd", bufs=2, space="PSUM")
        pd = pd_cm.__enter__()
        po_cm = tc.tile_pool(name="po", bufs=1, space="PSUM")
        po = po_cm.__enter__()
        p_out = po.tile([2, NH], f32, tag="po")
        for k in range(KD):
            for (stk, mv, bp, dstT) in ((stk_h, mov_h, t_hbp, hsT),
                                        (stk_t, mov_t, t_tbp, tsT)):
                p_hs = ph.tile([128, NH], f32, tag="ph")
                nc.tensor.matmul(p_hs[:], stk[0:96, k, :], mv[:])
                nc.scalar.activation(dstT[:, k, :], p_hs[:],
                                     AF.Tanh, bias=bp[:, k:k + 1])
            for half in range(2):
                g = 2 * k + half
                rows = slice(half * 64, (half + 1) * 64)
                p_u = pd.tile([128, NH], f32, tag="pd")
                nc.tensor.matmul(p_u[:], t_wdec[rows, g, :], tsT[rows, k, :])
                v = sbt.tile([128, NH], bf16, tag="v")
                nc.vector.tensor_mul(v[0:64, :], p_u[0:64, :], hsT[rows, k, :])
                nc.vector.tensor_mul(v[64:128, :], p_u[64:128, :], hsT[rows, k, :])
                nc.tensor.matmul(p_out[:], t_smat, v[:],
                                 start=(g == 0), stop=(g == G - 1))
        out_sb = sbt.tile([2, NH], f32, tag="out")
        nc.scalar.activation(out_sb[:], p_out[:], AF.Identity, bias=t_decb)
        nc.sync.dma_start(y[:], out_sb[:])
        po_cm.__exit__(None, None, None)
        pd_cm.__exit__(None, None, None)
        ph_cm.__exit__(None, None, None)

    nc.compile()
    return nc


def _wrap16(idx, n_slots):
    """int16 index layout for gpsimd gathers: wrapped in 16 partitions,
    replicated across the 8 gpsimd cores."""
    out = np.zeros((128, n_slots), np.int16)
    for j, v in enumerate(idx):
        out[np.arange(8) * 16 + j % 16, j // 16] = v
    return out


def _bf(a):
    import ml_dtypes
    return np.asarray(a, np.float32).astype(ml_dtypes.bfloat16)


def _fill(lay, ncols, rows, dtype, vals):
    out = np.zeros((rows, ncols), dtype=dtype)
    for name, arr in vals.items():
        r0, r, c0, c = lay[name]
        a = np.asarray(arr)
        if a.ndim != 2:
            a = a.reshape(r, c)
        out[r0:r0 + a.shape[0], c0:c0 + a.shape[1]] = a
    return out


def pack_inputs(inputs):
    import ml_dtypes
    bfd = ml_dtypes.bfloat16
    x = np.asarray(inputs["x"], np.float32)
    entity_pos = np.asarray(inputs["entity_pos"])
    hts = np.asarray(inputs["hts"])

    def W(name):
        return np.asarray(inputs[name], np.float32)

    head_w, tail_w = W("head_w"), W("tail_w")
    fin_w = W("fin_w").reshape(OUT_CH, 64)
    fin_b = W("fin_b")
    w2h_f = fin_w.T @ head_w[D:]          # [64, 768]
    w2t_f = fin_w.T @ tail_w[D:]
    hb_f = W("head_b") + fin_b @ head_w[D:]
    tb_f = W("tail_b") + fin_b @ tail_w[D:]

    a64 = _fill(LAY_A64, NC_A64, 64, bfd, {
        "ident": _bf(np.eye(NE)),
        "enc1w3": _bf(W("enc1_w").reshape(64, 3, 3).transpose(2, 1, 0).reshape(3, 192)),
        "enc2w": _bf(W("enc2_w").reshape(128, 64, 9).transpose(1, 2, 0).reshape(64, 1152)),
        "dec1wb": _bf(W("dec1_w").reshape(64, 192, 9).transpose(1, 2, 0)[128:].reshape(64, 576)),
        "ag1wx": _bf(W("ag1_wx").reshape(64, 64).T),
        "ag1ps": _bf(W("ag1_psi").reshape(1, 64).T),
    })
    bott_c = _fill(LAY_BOTT, NC_BOTT, 128, bfd, {
        "bott": _bf(W("bott_w").reshape(256, 128, 9).transpose(1, 2, 0).reshape(128, 2304)),
    })
    ag2_c = _fill(LAY_AG2, NC_AG2, 128, bfd, {
        "ag2wg": _bf(W("ag2_wg").reshape(128, 256).T.reshape(2, 128, 128)
                     .transpose(1, 0, 2).reshape(128, 256)),
        "ag2wx": _bf(W("ag2_wx").reshape(128, 128).T),
        "ag2ps": _bf(W("ag2_psi").reshape(1, 128).T),
    })
    d2w = W("dec2_w").reshape(128, 384, 9).transpose(1, 2, 0)\
        .reshape(3, 128, 9, 128).transpose(1, 0, 2, 3)       # [128, kc, 9, 128]
    dec2a_c = _fill(LAY_DEC2A, NC_DEC2A, 128, bfd,
                    {"dec2a": _bf(d2w[:, 0:2].reshape(128, 2304))})
    dec2b_c = _fill(LAY_DEC2B, NC_DEC2B, 128, bfd,
                    {"dec2b": _bf(d2w[:, 2].reshape(128, 1152))})
    b128 = _fill(LAY_B128, NC_B128, 128, bfd, {
        "dec1wa": _bf(W("dec1_w").reshape(64, 192, 9).transpose(1, 2, 0)[:128]
                      .reshape(128, 576)),
        "ag1wg": _bf(W("ag1_wg").reshape(64, 128).T),
    })
    w1h = _bf(head_w[:D].reshape(KD, 128, D).transpose(1, 0, 2).reshape(128, KD * D))
    w1t = _bf(tail_w[:D].reshape(KD, 128, D).transpose(1, 0, 2).reshape(128, KD * D))

    stk_h = np.zeros((128, KD * 128), np.float32)
    stk_h[0:64] = w2h_f.reshape(64, KD, 128).reshape(64, KD * 128)
    stk_t = np.zeros((128, KD * 128), np.float32)
    stk_t[0:64] = w2t_f.reshape(64, KD, 128).reshape(64, KD * 128)
    wd = W("decoder_w").reshape(G, 64, 64, 2).transpose(2, 0, 3, 1).reshape(64, G * 128)
    smat = np.zeros((128, 2), np.float32)
    smat[:64, 0] = 1.0
    smat[64:, 1] = 1.0
    f_c = _fill(LAY_F, NC_F, 128, bfd, {
        "stk_h": _bf(stk_h),
        "stk_t": _bf(stk_t),
        "wdec": _bf(np.concatenate([wd, wd], axis=0)),
        "smat": _bf(smat),
    })

    mf = _fill(LAY_MF, NC_MF, 128, np.float32, {
        "enc1b": W("enc1_b").reshape(64, 1),
        "enc2b": W("enc2_b").reshape(128, 1),
        "bottb": W("bott_b").reshape(2, 128).T,
        "dec2b_b": W("dec2_b").reshape(128, 1),
        "dec1b": W("dec1_b").reshape(64, 1),
        "hbp": hb_f.reshape(KD, 128).T,
        "tbp": tb_f.reshape(KD, 128).T,
        "decb": W("decoder_b").reshape(2, 1),
        # emask filled per-core below
    })

    shared = dict(a64=a64, bott_c=bott_c, ag2_c=ag2_c, dec2a_c=dec2a_c,
                  dec2b_c=dec2b_c, b128=b128, w1h=w1h, w1t=w1t, f_c=f_c)

    in_maps = []
    for c in range(NCORES):
        b, h = c // 2, c % 2
        m = dict(shared)
        m["x_b"] = np.ascontiguousarray(x[b])
        start = entity_pos[b, :, 0].astype(np.int64)
        idx = np.minimum(start + 1, L - 1).astype(np.int16)
        m["eidx"] = _wrap16(np.tile(idx, 4), 8)
        mfc = mf.copy()
        r0, r, c0, cc = LAY_MF["emask"]
        mfc[r0:r0 + NE, c0] = (start + 1 < L).astype(np.float32)
        m["mf"] = mfc
        hi = hts[b, h * NH:(h + 1) * NH, 0].astype(np.int64)
        ti = hts[b, h * NH:(h + 1) * NH, 1].astype(np.int64)
        ohm = np.zeros((64, NH), np.float32)
        ohm[hi, np.arange(NH)] = 1.0
        ohm[32 + ti, np.arange(NH)] = 1.0
        m["ohm"] = ohm.astype(bfd)
        m["pidx"] = _wrap16((hi * NE + ti).astype(np.int16), NH // 16)
        in_maps.append(m)
    return in_maps


_NC_CACHE = None


def get_nc():
    global _NC_CACHE
    if _NC_CACHE is None:
        _NC_CACHE = build_nc()
    return _NC_CACHE


def kernel(**inputs):
    nc = get_nc()
    in_maps = pack_inputs(inputs)
    res = run_bass_kernel_spmd(nc, in_maps, core_ids=list(range(NCORES)))
    out = np.empty((B * P, 2), np.float32)
    for c in range(NCORES):
        b, h = c // 2, c % 2
        yc = res.results[c]["y"]                  # [2, NH]
        out[b * P + h * NH:b * P + (h + 1) * NH, :] = yc.T
    return out


# revision 10
# speedup vs baseline: 1.6314x; 1.2510x over previous
"""Trainium2 Bass kernel for nn_CoreferenceResolver (coref UNet + pair decoder).

Sharding: core c handles batch b=c//2 and pair-half h=c%2 (496 of 992 pairs).
The gather/cosine/UNet stages are replicated on the two cores sharing a batch;
the extractor linears and group-bilinear decoder are sharded over pairs.

v1 design notes (vs the f32r baseline):
- all weights/activations bf16 (halves DMA bytes, PE still 1 cycle/row)
- fin 1x1 conv + amap gathers + W2 matmuls folded into host-precomputed
  W2' = fin_w^T @ head_w[768:] and a single d1 gather
- extractor = one stacked K=128 matmul per (k, extractor):
  mov rows 0:64 d1[pairs], 64:96 one-hot(hi) (host), 96:128 one-hot(ti)
- enc1 via 3 column-shifted padded images (K=3 matmuls, 6 total)
- weights arrive as a few packed DMA chunks ordered just-in-time so the
  cos->image DMA never queues behind megabytes of weight traffic
- PE warmup chain holds the p-state ramp so real matmuls price at full speed
"""
import os
import sys

for _p in ("/opt/trn_rl_repo",):
    if os.path.isdir(_p) and _p not in sys.path:
        sys.path.insert(0, _p)

import numpy as np

import concourse.bass as bass
import concourse.tile as tile
from concourse import bacc, mybir
from concourse.bass_utils import run_bass_kernel_spmd

f32 = mybir.dt.float32
i16 = mybir.dt.int16
bf16 = mybir.dt.bfloat16
AF = mybir.ActivationFunctionType
OP = mybir.AluOpType

B, L, D, H = 4, 1024, 768, 12
NE, P = 32, 992
BLOCK = 64
G = D // BLOCK          # 12 groups
OUT_CH = 256
NCORES = 8
NH = P // 2             # 496 pairs per core
KD = D // 128           # 6 chunks of the D dim

# ---------------------------------------------------------------------------
# packed-chunk layouts (shapes only; shared by build_nc and pack_inputs)
# entries: name -> (row0, rows, col0, cols); each chunk = one DRAM tensor.
# ---------------------------------------------------------------------------


def _mklayout(rows, entries):
    lay, col = {}, 0
    for name, r0, r, c in entries:
        lay[name] = (r0, r, col, c)
        col += c
    return lay, col


LAY_A64, NC_A64 = _mklayout(64, [
    ("ident", 0, 32, 32),
    ("enc1w3", 0, 3, 192),          # [dx, dy*64+c]
    ("enc2w", 0, 64, 1152),         # [64, 9*128]
    ("dec1wb", 0, 64, 576),         # [64, 9*64]
    ("ag1wx", 0, 64, 64),
    ("ag1ps", 0, 64, 1),
])
LAY_BOTT, NC_BOTT = _mklayout(128, [("bott", 0, 128, 2304)])   # [128, 9*256]
LAY_AG2, NC_AG2 = _mklayout(128, [
    ("ag2wg", 0, 128, 256),         # [128, 2*128]
    ("ag2wx", 0, 128, 128),
    ("ag2ps", 0, 128, 1),
])
LAY_DEC2A, NC_DEC2A = _mklayout(128, [("dec2a", 0, 128, 2304)])  # kc 0:2
LAY_DEC2B, NC_DEC2B = _mklayout(128, [("dec2b", 0, 128, 1152)])  # kc 2
LAY_B128, NC_B128 = _mklayout(128, [
    ("dec1wa", 0, 128, 576),        # [128, 9*64]
    ("ag1wg", 0, 128, 64),
])
LAY_F, NC_F = _mklayout(128, [
    ("stk_h", 0, 128, 768),         # rows 0:64 W2h'; 64:96 <- EW1 (device)
    ("stk_t", 0, 128, 768),         # rows 0:64 W2t'; 64:96 <- EW1t (device)
    ("wdecA", 0, 128, 768),         # blockdiag per k-chunk, o=0
    ("wdecB", 0, 128, 768),         # blockdiag per k-chunk, o=1
    ("ssum", 0, 128, 4),            # [all-ones|0 ; 0|all-ones] selectors
])
LAY_MF, NC_MF = _mklayout(128, [
    ("emask", 0, 32, 1),
    ("enc1b", 0, 64, 1),
    ("enc2b", 0, 128, 1),
    ("bottb", 0, 128, 2),
    ("dec2b_b", 0, 128, 1),
    ("dec1b", 0, 64, 1),
    ("hbp", 0, 128, 6),
    ("tbp", 0, 128, 6),
    ("decb", 0, 2, 1),
])


def build_nc():
    nc = bacc.Bacc("TRN2", target_bir_lowering=False, debug=False, num_devices=NCORES)

    def inp(name, shape, dt=f32):
        return nc.dram_tensor(name, shape, dt, kind="ExternalInput")

    x_b = inp("x_b", [L, D])
    eidx_d = inp("eidx", [128, 8], i16)
    pidx_d = inp("pidx", [128, NH // 16], i16)
    ohm_d = inp("ohm", [64, NH], bf16)
    mf_d = inp("mf", [128, NC_MF])
    a64_d = inp("a64", [64, NC_A64], bf16)
    bott_d = inp("bott_c", [128, NC_BOTT], bf16)
    ag2_d = inp("ag2_c", [128, NC_AG2], bf16)
    dec2a_d = inp("dec2a_c", [128, NC_DEC2A], bf16)
    dec2b_d = inp("dec2b_c", [128, NC_DEC2B], bf16)
    b128_d = inp("b128", [128, NC_B128], bf16)
    w1h_d = inp("w1h", [128, KD * D], bf16)
    w1t_d = inp("w1t", [128, KD * D], bf16)
    f_d = inp("f_c", [128, NC_F], bf16)

    y = nc.dram_tensor("y", [2, NH], f32, kind="ExternalOutput")

    from contextlib import ExitStack
    with tile.TileContext(nc) as tc, ExitStack() as _ctx:
        sbw = _ctx.enter_context(tc.tile_pool(name="sbw", bufs=1))   # persistent
        sbt = _ctx.enter_context(tc.tile_pool(name="sbt", bufs=3))   # rotating temps
        pu_cm = tc.tile_pool(name="pu", bufs=3, space="PSUM")
        pu = pu_cm.__enter__()

        # ---------------- t0: gpsimd: eidx, gather, warm tile, memsets -----
        t_eidx = sbw.tile([128, 8], i16, tag="eidx")
        nc.gpsimd.dma_start(t_eidx[:], eidx_d[:])
        warm = sbw.tile([1, 512], bf16, tag="warm")
        nc.gpsimd.memset(warm[:], 0.0)
        # entities replicated on partition blocks 0:32 / 32:64 / 64:96 / 96:128
        ent_raw = sbw.tile([128, 1, D], f32, tag="entraw")
        nc.gpsimd.dma_gather(ent_raw[:], x_b[:], t_eidx[:],
                             num_idxs=128, num_idxs_reg=128, elem_size=D)
        ent = ent_raw[0:NE, 0, :]

        # padded intermediates (bf16) + border-only memsets
        img3 = sbw.tile([3, 34 * 34], bf16, tag="img3")
        c1p = sbw.tile([64, 34 * 34], bf16, tag="c1p")
        p1p = sbw.tile([64, 18 * 18], bf16, tag="p1p")
        c2p = sbw.tile([128, 18 * 18], bf16, tag="c2p")
        p2p = sbw.tile([128, 10 * 10], bf16, tag="p2p")
        u2p0 = sbw.tile([128, 18 * 18], bf16, tag="u2p0")
        u2p1 = sbw.tile([128, 18 * 18], bf16, tag="u2p1")
        att2p = sbw.tile([128, 18 * 18], bf16, tag="att2p")
        u1p = sbw.tile([128, 34 * 34], bf16, tag="u1p")
        att1p = sbw.tile([64, 34 * 34], bf16, tag="att1p")

        nc.gpsimd.memset(img3[:], 0.0)
        ones_bf = sbw.tile([1, 128], bf16, tag="ones")
        nc.gpsimd.memset(ones_bf[:], 1.0)

        def borders(t, n):
            v = t[:].rearrange("c (h w) -> c h w", h=n, w=n)
            nc.gpsimd.memset(v[:, 0:n:n - 1, :], 0.0)
            nc.gpsimd.memset(v[:, :, 0:n:n - 1], 0.0)

        for t, n in ((c1p, 34), (p1p, 18), (c2p, 18), (p2p, 10), (u2p0, 18),
                     (u2p1, 18), (att2p, 18), (u1p, 34), (att1p, 34)):
            borders(t, n)

        # ---------------- sync-engine DMA chunks (just-in-time order) ------
        def load(dram, shape, dt, tag, eng=None):
            t = sbw.tile(shape, dt, tag=tag)
            (eng or nc.sync).dma_start(t[:], dram[:])
            return t

        t_mf = load(mf_d, [128, NC_MF], f32, "mf")
        t_a64 = load(a64_d, [64, NC_A64], bf16, "a64")
        t_bott = load(bott_d, [128, NC_BOTT], bf16, "bott")
        t_ag2 = load(ag2_d, [128, NC_AG2], bf16, "ag2")
        t_pidx = load(pidx_d, [128, NH // 16], i16, "pidx")

        mov_h = sbw.tile([96, NH], bf16, tag="movh")
        mov_t = sbw.tile([96, NH], bf16, tag="movt")
        nc.sync.dma_start(mov_h[64:96, :], ohm_d[0:32, :])
        nc.sync.dma_start(mov_t[64:96, :], ohm_d[32:64, :])

        def vw(tile_, lay, name, shape=None):
            row0, r, c0, c = lay[name]
            ap = tile_[row0:row0 + r, c0:c0 + c]
            if shape is not None and len(shape) > 2:
                pat = {3: "p (a b) -> p a b", 4: "p (a b c) -> p a b c"}[len(shape)]
                kw = dict(zip("abc", shape[1:]))
                ap = ap.rearrange(pat, **kw)
            return ap

        t_ident = vw(t_a64, LAY_A64, "ident")
        t_enc1w = vw(t_a64, LAY_A64, "enc1w3", (3, 3, 64))
        t_enc2w = vw(t_a64, LAY_A64, "enc2w", (64, 9, 128))
        t_dec1wb = vw(t_a64, LAY_A64, "dec1wb", (64, 9, 64))
        t_ag1wx = vw(t_a64, LAY_A64, "ag1wx")
        t_ag1ps = vw(t_a64, LAY_A64, "ag1ps")
        t_bottw = vw(t_bott, LAY_BOTT, "bott", (128, 9, 256))
        t_ag2wg = vw(t_ag2, LAY_AG2, "ag2wg", (128, 2, 128))
        t_ag2wx = vw(t_ag2, LAY_AG2, "ag2wx")
        t_ag2ps = vw(t_ag2, LAY_AG2, "ag2ps")

        t_emask = vw(t_mf, LAY_MF, "emask")
        t_enc1b = vw(t_mf, LAY_MF, "enc1b")
        t_enc2b = vw(t_mf, LAY_MF, "enc2b")
        t_bottb = vw(t_mf, LAY_MF, "bottb")
        t_dec2bb = vw(t_mf, LAY_MF, "dec2b_b")
        t_dec1b = vw(t_mf, LAY_MF, "dec1b")
        t_hbp = vw(t_mf, LAY_MF, "hbp")
        t_tbp = vw(t_mf, LAY_MF, "tbp")
        t_decb = vw(t_mf, LAY_MF, "decb")

        # ---------------- PE warmup chain (p-state ramp) -------------------
        p_warm = pu.tile([1, 512], f32, tag="pu")
        NWARM = 5
        for i in range(NWARM):
            nc.tensor.matmul(p_warm[:], warm[0:1, 0:1], warm[:],
                             start=(i == 0), stop=(i == NWARM - 1))

        # ---------------- front-end: norms + transposes + cos --------------
        # dummy sqrt at t0 -> the preamble table load covers Sqrt+Square
        dummy = sbw.tile([1, 2], f32, tag="dum")
        nc.scalar.activation(dummy[:, 0:1], warm[0:1, 0:1], AF.Sqrt)
        sq_scr = sbt.tile([128, D], bf16, tag="t")
        ss = sbw.tile([128, 1], f32, tag="ss")
        nc.scalar.activation(sq_scr[0:96, :], ent_raw[0:96, 0, :], AF.Square,
                             accum_out=ss[0:96, :])
        normc = sbw.tile([128, 1], f32, tag="normc")
        nc.scalar.activation(normc[0:96, :], ss[0:96, :], AF.Sqrt)
        # dummy sigmoid -> hoist the 2nd act-table load off the critical path
        nc.scalar.activation(dummy[:, 1:2], ss[0:1, :], AF.Sigmoid)
        nc.vector.tensor_single_scalar(normc[0:96, :], normc[0:96, :], 1e-13, op=OP.max)
        rinv = sbw.tile([NE, 1], f32, tag="rinv")
        nc.vector.reciprocal(rinv[:], normc[0:NE, :])
        nc.vector.tensor_tensor(out=rinv[:], in0=rinv[:], in1=t_emask, op=OP.mult)
        nrm = sbw.tile([NE, D], bf16, tag="nrm")
        nc.vector.tensor_scalar(out=nrm[:], in0=ent, scalar1=rinv[:],
                                scalar2=None, op0=OP.mult)

        nrmT = sbw.tile([128, KD, NE], bf16, tag="nrmT")
        for k in range(KD):
            p_t = pu.tile([128, NE], bf16, tag="pu")
            nc.tensor.transpose(p_t[:], nrm[:, k * 128:(k + 1) * 128], t_ident)
            nc.vector.tensor_copy(nrmT[:, k, :], p_t[:])

        p_cos = pu.tile([NE, NE], f32, tag="pu")
        for k in range(KD):
            nc.tensor.matmul(p_cos[:], nrmT[:, k, :], nrmT[:, k, :],
                             start=(k == 0), stop=(k == KD - 1))
        s_cos = sbw.tile([NE, NE], bf16, tag="scos")
        nc.vector.tensor_copy(s_cos[:], p_cos[:])

        # ---------------- image staging: 3 column-shifted padded copies ----
        img3v = img3[:].rearrange("c (h w) -> c h w", h=34, w=34)
        nc.sync.dma_start(img3v[0:1, 1:33, 2:34], s_cos[:])
        nc.scalar.dma_start(img3v[1:2, 1:33, 1:33], s_cos[:])
        nc.gpsimd.dma_start(img3v[2:3, 1:33, 0:32], s_cos[:])

        # remaining weight chunks: a tiny token copy (reads s_cos) makes each
        # chunk DMA wait until the front-end is done with the DMA device
        def loadT(dram, shape, dt, tag):
            t = sbw.tile(shape, dt, tag=tag)
            nc.vector.tensor_copy(t[0:1, 0:1], s_cos[0:1, 0:1])
            nc.sync.dma_start(t[:], dram[:])
            return t

        t_dec2wa = loadT(dec2a_d, [128, NC_DEC2A], bf16, "dec2a")
        t_dec2wa = t_dec2wa[:].rearrange("p (a b c) -> p a b c", a=2, b=9, c=128)
        t_dec2wb = loadT(dec2b_d, [128, NC_DEC2B], bf16, "dec2b")
        t_dec2wb = t_dec2wb[:].rearrange("p (b c) -> p b c", b=9, c=128)
        t_w1h = loadT(w1h_d, [128, KD * D], bf16, "w1h")
        t_w1h = t_w1h[:].rearrange("p (k d) -> p k d", k=KD)
        t_b128 = loadT(b128_d, [128, NC_B128], bf16, "b128")
        t_dec1wa = vw(t_b128, LAY_B128, "dec1wa", (128, 9, 64))
        t_ag1wg = vw(t_b128, LAY_B128, "ag1wg")
        t_w1t = loadT(w1t_d, [128, KD * D], bf16, "w1t")
        t_w1t = t_w1t[:].rearrange("p (k d) -> p k d", k=KD)
        t_f = loadT(f_d, [128, NC_F], bf16, "f")
        stk_h = vw(t_f, LAY_F, "stk_h", (128, KD, 128))
        stk_t = vw(t_f, LAY_F, "stk_t", (128, KD, 128))
        t_wdecA = vw(t_f, LAY_F, "wdecA", (128, KD, 128))
        t_wdecB = vw(t_f, LAY_F, "wdecB", (128, KD, 128))
        t_ssum = vw(t_f, LAY_F, "ssum")

        # ---------------- enc1: 2 halves x 3 dy matmuls (K=3) --------------
        c1pv = c1p[:].rearrange("c (h w) -> c h w", h=34, w=34)
        p_c1 = pu.tile([64, 1024], f32, tag="pu")
        for hh in range(2):
            for dy in range(3):
                rows = slice(dy + 16 * hh, dy + 16 * hh + 16)
                nc.tensor.matmul(p_c1[:, hh * 512:(hh + 1) * 512],
                                 t_enc1w[:, dy, :],
                                 img3v[0:3, rows, 1:33],
                                 start=(dy == 0), stop=(dy == 2))
        for hh in range(2):
            nc.scalar.activation(c1pv[:, 1 + 16 * hh:17 + 16 * hh, 1:33],
                                 p_c1[:, hh * 512:(hh + 1) * 512].rearrange(
                                     "c (h w) -> c h w", h=16, w=32),
                                 AF.Relu, bias=t_enc1b)

        # ---------------- pool1 (gpsimd) -----------------------------------
        p1pv = p1p[:].rearrange("c (h w) -> c h w", h=18, w=18)
        tmp = sbt.tile([64, 16, 16], bf16, tag="t")
        nc.vector.tensor_max(tmp[:], c1pv[:, 1:33:2, 1:33:2], c1pv[:, 1:33:2, 2:34:2])
        nc.vector.tensor_max(tmp[:], tmp[:], c1pv[:, 2:34:2, 1:33:2])
        nc.vector.tensor_max(p1pv[:, 1:17, 1:17], tmp[:], c1pv[:, 2:34:2, 2:34:2])

        # ---------------- enc2: 9 taps K=64 --------------------------------
        p_c2 = pu.tile([128, 256], f32, tag="pu")
        for tap in range(9):
            dy, dx = tap // 3, tap % 3
            nc.tensor.matmul(p_c2[:], t_enc2w[:, tap, :],
                             p1pv[:, dy:dy + 16, dx:dx + 16],
                             start=(tap == 0), stop=(tap == 8))
        c2pv = c2p[:].rearrange("c (h w) -> c h w", h=18, w=18)
        nc.scalar.activation(c2pv[:, 1:17, 1:17],
                             p_c2[:].rearrange("c (h w) -> c h w", h=16, w=16),
                             AF.Relu, bias=t_enc2b)

        # ---------------- pool2 (gpsimd) -----------------------------------
        p2pv = p2p[:].rearrange("c (h w) -> c h w", h=10, w=10)
        tmp2 = sbt.tile([128, 8, 8], bf16, tag="t")
        nc.vector.tensor_max(tmp2[:], c2pv[:, 1:17:2, 1:17:2], c2pv[:, 1:17:2, 2:18:2])
        nc.vector.tensor_max(tmp2[:], tmp2[:], c2pv[:, 2:18:2, 1:17:2])
        nc.vector.tensor_max(p2pv[:, 1:9, 1:9], tmp2[:], c2pv[:, 2:18:2, 2:18:2])

        # ---------------- bottleneck: 9 taps x 2 M-chunks, K=128 -----------
        c3 = []
        for mc in range(2):
            p_c3 = pu.tile([128, 64], f32, tag="pu")
            for tap in range(9):
                dy, dx = tap // 3, tap % 3
                nc.tensor.matmul(p_c3[:], t_bottw[:, tap, mc * 128:(mc + 1) * 128],
                                 p2pv[:, dy:dy + 8, dx:dx + 8],
                                 start=(tap == 0), stop=(tap == 8))
            c3s = sbt.tile([128, 8, 8], bf16, tag=f"c3_{mc}")
            nc.scalar.activation(c3s[:], p_c3[:].rearrange("c (h w) -> c h w", h=8, w=8),
                                 AF.Relu, bias=t_bottb[:, mc:mc + 1])
            c3.append(c3s)

        # ---------------- up2 ----------------------------------------------
        u2p0v = u2p0[:].rearrange("c (h w) -> c h w", h=18, w=18)
        u2p1v = u2p1[:].rearrange("c (h w) -> c h w", h=18, w=18)
        for src, dv in ((c3[0], u2p0v), (c3[1], u2p1v)):
            for i in range(2):
                for j in range(2):
                    nc.vector.tensor_copy(dv[:, 1 + i:17:2, 1 + j:17:2], src[:])

        # ---------------- attention gate 2 + dec2 (interleaved) ------------
        # the 18 u2-taps of dec2 fill the PE while the psi/sigmoid chain of
        # the attention gate bounces between ACT and DVE
        p_a2 = pu.tile([128, 256], f32, tag="pu")
        nc.tensor.matmul(p_a2[:], t_ag2wg[:, 0, :], u2p0v[:, 1:17, 1:17],
                         start=True, stop=False)
        nc.tensor.matmul(p_a2[:], t_ag2wg[:, 1, :], u2p1v[:, 1:17, 1:17],
                         start=False, stop=False)
        nc.tensor.matmul(p_a2[:], t_ag2wx, c2pv[:, 1:17, 1:17],
                         start=False, stop=True)
        p_d2 = pu.tile([128, 256], f32, tag="pu")
        n_mm = 0
        for kc in range(2):
            src = (u2p0v, u2p1v)[kc]
            for tap in range(9):
                dy, dx = tap // 3, tap % 3
                nc.tensor.matmul(p_d2[:], t_dec2wa[:, kc, tap, :],
                                 src[:, dy:dy + 16, dx:dx + 16],
                                 start=(n_mm == 0), stop=False)
                n_mm += 1
        r2 = sbt.tile([128, 256], bf16, tag="t")
        nc.scalar.activation(r2[:], p_a2[:], AF.Relu)
        p_g2 = pu.tile([1, 256], f32, tag="pu")
        nc.tensor.matmul(p_g2[:], t_ag2ps, r2[:])
        a2 = sbt.tile([1, 256], bf16, tag="a2")
        nc.scalar.activation(a2[:], p_g2[:], AF.Sigmoid)
        p_a2b = pu.tile([128, 256], f32, tag="pu")
        nc.tensor.matmul(p_a2b[:], ones_bf[:], a2[:])
        att2pv = att2p[:].rearrange("c (h w) -> c h w", h=18, w=18)
        nc.vector.tensor_mul(att2pv[:, 1:17, 1:17],
                             p_a2b[:].rearrange("c (h w) -> c h w", h=16, w=16),
                             c2pv[:, 1:17, 1:17])
        for tap in range(9):
            dy, dx = tap // 3, tap % 3
            nc.tensor.matmul(p_d2[:], t_dec2wb[:, tap, :],
                             att2pv[:, dy:dy + 16, dx:dx + 16],
                             start=False, stop=(tap == 8))
        d2s = sbt.tile([128, 256], bf16, tag="d2s")
        nc.scalar.activation(d2s[:], p_d2[:], AF.Relu, bias=t_dec2bb)

        # ---------------- up1 ----------------------------------------------
        u1pv = u1p[:].rearrange("c (h w) -> c h w", h=34, w=34)
        d2v = d2s[:].rearrange("c (h w) -> c h w", h=16, w=16)
        for i in range(2):
            for j in range(2):
                nc.vector.tensor_copy(u1pv[:, 1 + i:33:2, 1 + j:33:2], d2v[:])

        # ---------------- attention gate 1 + dec1 + EW (interleaved) -------
        # dec1's u1-taps and the EW premultiplies fill the PE while the
        # psi/sigmoid chain runs; att1-taps close the dec1 groups afterwards
        p_a1 = pu.tile([64, 1024], f32, tag="pu")
        for hh in range(2):
            rows = slice(1 + 16 * hh, 17 + 16 * hh)
            nc.tensor.matmul(p_a1[:, hh * 512:(hh + 1) * 512], t_ag1wx,
                             c1pv[:, rows, 1:33], start=True, stop=False)
            nc.tensor.matmul(p_a1[:, hh * 512:(hh + 1) * 512], t_ag1wg,
                             u1pv[:, rows, 1:33], start=False, stop=True)
        d1 = sbw.tile([64, 1024], f32, tag="d1")
        p_d1 = pu.tile([64, 1024], f32, tag="pu")

        def dec1_taps(hh, wtile, srcv, start):
            cols = slice(hh * 512, (hh + 1) * 512)
            for tap in range(9):
                dy, dx = tap // 3, tap % 3
                rows = slice(dy + 16 * hh, dy + 16 * hh + 16)
                nc.tensor.matmul(p_d1[:, cols], wtile[:, tap, :],
                                 srcv[:, rows, dx:dx + 32],
                                 start=(start and tap == 0),
                                 stop=(not start and tap == 8))

        dec1_taps(0, t_dec1wa, u1pv, True)
        r1 = sbt.tile([64, 1024], bf16, tag="t")
        nc.scalar.activation(r1[:], p_a1[:], AF.Relu)
        p_g1 = pu.tile([1, 1024], f32, tag="pu")
        for hh in range(2):
            nc.tensor.matmul(p_g1[:, hh * 512:(hh + 1) * 512], t_ag1ps,
                             r1[:, hh * 512:(hh + 1) * 512])
        dec1_taps(1, t_dec1wa, u1pv, True)
        a1 = sbt.tile([1, 1024], bf16, tag="a1")
        nc.scalar.activation(a1[:], p_g1[:], AF.Sigmoid)
        p_a1b = pu.tile([64, 1024], f32, tag="pu")
        for hh in range(2):
            nc.tensor.matmul(p_a1b[:, hh * 512:(hh + 1) * 512], ones_bf[:, :64],
                             a1[:, hh * 512:(hh + 1) * 512])
        att1pv = att1p[:].rearrange("c (h w) -> c h w", h=34, w=34)
        nc.vector.tensor_mul(att1pv[:, 1:33, 1:33],
                             p_a1b[:].rearrange("c (h w) -> c h w", h=32, w=32),
                             c1pv[:, 1:33, 1:33])
        dec1_taps(0, t_dec1wb, att1pv, False)
        # EW-head premultiply: (ent @ head_w[:768]) at rows 64:96
        p_ewh = pu.tile([128, D], f32, tag="pu")
        for k in range(KD):
            for n0, n1 in ((0, 512), (512, D)):
                nc.tensor.matmul(p_ewh[64:96, n0:n1], nrmT[:, k, :],
                                 t_w1h[:, k, n0:n1],
                                 start=(k == 0), stop=(k == KD - 1))
        nc.scalar.activation(stk_h[64:96, :, :].rearrange("p a b -> p (a b)"),
                             p_ewh[64:96, :], AF.Copy, scale=normc[64:96, :])
        dec1_taps(1, t_dec1wb, att1pv, False)
        nc.scalar.activation(d1[:], p_d1[:], AF.Relu, bias=t_dec1b)

        p_ewt = pu.tile([128, D], f32, tag="pu")
        for k in range(KD):
            for n0, n1 in ((0, 512), (512, D)):
                nc.tensor.matmul(p_ewt[64:96, n0:n1], nrmT[:, k, :],
                                 t_w1t[:, k, n0:n1],
                                 start=(k == 0), stop=(k == KD - 1))
        nc.scalar.activation(stk_t[64:96, :, :].rearrange("p a b -> p (a b)"),
                             p_ewt[64:96, :], AF.Copy, scale=normc[64:96, :])

        # ---------------- d1 gather -> mov rows 0:64 ------------------------
        d1g = sbt.tile([64, NH], f32, tag="d1g")
        nc.gpsimd.ap_gather(d1g[:].rearrange("c (n o) -> c n o", o=1),
                            d1[:].rearrange("c (n o) -> c n o", o=1), t_pidx[:],
                            channels=64, num_elems=1024, d=1, num_idxs=NH)
        nc.vector.tensor_copy(mov_h[0:64, :], d1g[:])
        nc.vector.tensor_copy(mov_t[0:64, :], d1g[:])

        pu_cm.__exit__(None, None, None)

        # ---------------- pair features + decoder --------------------------
        hsT = sbw.tile([128, KD, NH], bf16, tag="hsT")
        tsT = sbw.tile([128, KD, NH], bf16, tag="tsT")
        ph_cm = tc.tile_pool(name="ph", bufs=4, space="PSUM")
        ph = ph_cm.__enter__()
        pd_cm = tc.tile_pool(name="pd", bufs=2, space="PSUM")
        pd = pd_cm.__enter__()
        po_cm = tc.tile_pool(name="po", bufs=1, space="PSUM")
        po = po_cm.__enter__()
        p_out = po.tile([2, NH], f32, tag="po")
        for k in range(KD):
            for (stk, mv, bp, dstT) in ((stk_h, mov_h, t_hbp, hsT),
                                        (stk_t, mov_t, t_tbp, tsT)):
                p_hs = ph.tile([128, NH], f32, tag="ph")
                nc.tensor.matmul(p_hs[:], stk[0:96, k, :], mv[:])
                nc.scalar.activation(dstT[:, k, :], p_hs[:],
                                     AF.Tanh, bias=bp[:, k:k + 1])
            for half, wd in ((0, t_wdecA), (1, t_wdecB)):
                p_u = pd.tile([128, NH], f32, tag="pd")
                nc.tensor.matmul(p_u[:], wd[:, k, :], tsT[:, k, :])
                v = sbt.tile([128, NH], bf16, tag="v")
                nc.vector.tensor_mul(v[:], p_u[:], hsT[:, k, :])
                nc.tensor.matmul(p_out[:], t_ssum[:, 2 * half:2 * half + 2], v[:],
                                 start=(k == 0 and half == 0),
                                 stop=(k == KD - 1 and half == 1))
        out_sb = sbt.tile([2, NH], f32, tag="out")
        nc.scalar.activation(out_sb[:], p_out[:], AF.Identity, bias=t_decb)
        nc.sync.dma_start(y[:], out_sb[:])
        po_cm.__exit__(None, None, None)
        pd_cm.__exit__(None, None, None)
        ph_cm.__exit__(None, None, None)

    nc.compile()
    return nc


def _wrap16(idx, n_slots):
    """int16 index layout for gpsimd gathers: wrapped in 16 partitions,
    replicated across the 8 gpsimd cores."""
    out = np.zeros((128, n_slots), np.int16)
    for j, v in enumerate(idx):
        out[np.arange(8) * 16 + j % 16, j // 16] = v
    return out


def _bf(a):
    import ml_dtypes
    return np.asarray(a, np.float32).astype(ml_dtypes.bfloat16)


def _fill(lay, ncols, rows, dtype, vals):
    out = np.zeros((rows, ncols), dtype=dtype)
    for name, arr in vals.items():
        r0, r, c0, c = lay[name]
        a = np.asarray(arr)
        if a.ndim != 2:
            a = a.reshape(r, c)
        out[r0:r0 + a.shape[0], c0:c0 + a.shape[1]] = a
    return out


def pack_inputs(inputs):
    import ml_dtypes
    bfd = ml_dtypes.bfloat16
    x = np.asarray(inputs["x"], np.float32)
    entity_pos = np.asarray(inputs["entity_pos"])
    hts = np.asarray(inputs["hts"])

    def W(name):
        return np.asarray(inputs[name], np.float32)

    head_w, tail_w = W("head_w"), W("tail_w")
    fin_w = W("fin_w").reshape(OUT_CH, 64)
    fin_b = W("fin_b")
    w2h_f = fin_w.T @ head_w[D:]          # [64, 768]
    w2t_f = fin_w.T @ tail_w[D:]
    hb_f = W("head_b") + fin_b @ head_w[D:]
    tb_f = W("tail_b") + fin_b @ tail_w[D:]

    a64 = _fill(LAY_A64, NC_A64, 64, bfd, {
        "ident": _bf(np.eye(NE)),
        "enc1w3": _bf(W("enc1_w").reshape(64, 3, 3).transpose(2, 1, 0).reshape(3, 192)),
        "enc2w": _bf(W("enc2_w").reshape(128, 64, 9).transpose(1, 2, 0).reshape(64, 1152)),
        "dec1wb": _bf(W("dec1_w").reshape(64, 192, 9).transpose(1, 2, 0)[128:].reshape(64, 576)),
        "ag1wx": _bf(W("ag1_wx").reshape(64, 64).T),
        "ag1ps": _bf(W("ag1_psi").reshape(1, 64).T),
    })
    bott_c = _fill(LAY_BOTT, NC_BOTT, 128, bfd, {
        "bott": _bf(W("bott_w").reshape(256, 128, 9).transpose(1, 2, 0).reshape(128, 2304)),
    })
    ag2_c = _fill(LAY_AG2, NC_AG2, 128, bfd, {
        "ag2wg": _bf(W("ag2_wg").reshape(128, 256).T.reshape(2, 128, 128)
                     .transpose(1, 0, 2).reshape(128, 256)),
        "ag2wx": _bf(W("ag2_wx").reshape(128, 128).T),
        "ag2ps": _bf(W("ag2_psi").reshape(1, 128).T),
    })
    d2w = W("dec2_w").reshape(128, 384, 9).transpose(1, 2, 0)\
        .reshape(3, 128, 9, 128).transpose(1, 0, 2, 3)       # [128, kc, 9, 128]
    dec2a_c = _fill(LAY_DEC2A, NC_DEC2A, 128, bfd,
                    {"dec2a": _bf(d2w[:, 0:2].reshape(128, 2304))})
    dec2b_c = _fill(LAY_DEC2B, NC_DEC2B, 128, bfd,
                    {"dec2b": _bf(d2w[:, 2].reshape(128, 1152))})
    b128 = _fill(LAY_B128, NC_B128, 128, bfd, {
        "dec1wa": _bf(W("dec1_w").reshape(64, 192, 9).transpose(1, 2, 0)[:128]
                      .reshape(128, 576)),
        "ag1wg": _bf(W("ag1_wg").reshape(64, 128).T),
    })
    w1h = _bf(head_w[:D].reshape(KD, 128, D).transpose(1, 0, 2).reshape(128, KD * D))
    w1t = _bf(tail_w[:D].reshape(KD, 128, D).transpose(1, 0, 2).reshape(128, KD * D))

    stk_h = np.zeros((128, KD * 128), np.float32)
    stk_h[0:64] = w2h_f.reshape(64, KD, 128).reshape(64, KD * 128)
    stk_t = np.zeros((128, KD * 128), np.float32)
    stk_t[0:64] = w2t_f.reshape(64, KD, 128).reshape(64, KD * 128)
    wdq = W("decoder_w").reshape(G, 64, 64, 2)        # [g, i, j, o]
    wdA = np.zeros((128, KD, 128), np.float32)
    wdB = np.zeros((128, KD, 128), np.float32)
    for k in range(KD):
        wdA[0:64, k, 0:64] = wdq[2 * k, :, :, 0].T
        wdA[64:128, k, 64:128] = wdq[2 * k + 1, :, :, 0].T
        wdB[0:64, k, 0:64] = wdq[2 * k, :, :, 1].T
        wdB[64:128, k, 64:128] = wdq[2 * k + 1, :, :, 1].T
    ssum = np.zeros((128, 4), np.float32)
    ssum[:, 0] = 1.0
    ssum[:, 3] = 1.0
    f_c = _fill(LAY_F, NC_F, 128, bfd, {
        "stk_h": _bf(stk_h),
        "stk_t": _bf(stk_t),
        "wdecA": _bf(wdA.reshape(128, KD * 128)),
        "wdecB": _bf(wdB.reshape(128, KD * 128)),
        "ssum": _bf(ssum),
    })

    mf = _fill(LAY_MF, NC_MF, 128, np.float32, {
        "enc1b": W("enc1_b").reshape(64, 1),
        "enc2b": W("enc2_b").reshape(128, 1),
        "bottb": W("bott_b").reshape(2, 128).T,
        "dec2b_b": W("dec2_b").reshape(128, 1),
        "dec1b": W("dec1_b").reshape(64, 1),
        "hbp": hb_f.reshape(KD, 128).T,
        "tbp": tb_f.reshape(KD, 128).T,
        "decb": W("decoder_b").reshape(2, 1),
        # emask filled per-core below
    })

    shared = dict(a64=a64, bott_c=bott_c, ag2_c=ag2_c, dec2a_c=dec2a_c,
                  dec2b_c=dec2b_c, b128=b128, w1h=w1h, w1t=w1t, f_c=f_c)

    in_maps = []
    for c in range(NCORES):
        b, h = c // 2, c % 2
        m = dict(shared)
        m["x_b"] = np.ascontiguousarray(x[b])
        start = entity_pos[b, :, 0].astype(np.int64)
        idx = np.minimum(start + 1, L - 1).astype(np.int16)
        m["eidx"] = _wrap16(np.tile(idx, 4), 8)
        mfc = mf.copy()
        r0, r, c0, cc = LAY_MF["emask"]
        mfc[r0:r0 + NE, c0] = (start + 1 < L).astype(np.float32)
        m["mf"] = mfc
        hi = hts[b, h * NH:(h + 1) * NH, 0].astype(np.int64)
        ti = hts[b, h * NH:(h + 1) * NH, 1].astype(np.int64)
        ohm = np.zeros((64, NH), np.float32)
        ohm[hi, np.arange(NH)] = 1.0
        ohm[32 + ti, np.arange(NH)] = 1.0
        m["ohm"] = ohm.astype(bfd)
        m["pidx"] = _wrap16((hi * NE + ti).astype(np.int16), NH // 16)
        in_maps.append(m)
    return in_maps


_NC_CACHE = None


def get_nc():
    global _NC_CACHE
    if _NC_CACHE is None:
        _NC_CACHE = build_nc()
    return _NC_CACHE


def kernel(**inputs):
    nc = get_nc()
    in_maps = pack_inputs(inputs)
    res = run_bass_kernel_spmd(nc, in_maps, core_ids=list(range(NCORES)))
    out = np.empty((B * P, 2), np.float32)
    for c in range(NCORES):
        b, h = c // 2, c % 2
        yc = res.results[c]["y"]                  # [2, NH]
        out[b * P + h * NH:b * P + (h + 1) * NH, :] = yc.T
    return out


# revision 13
# speedup vs baseline: 1.8238x; 1.1179x over previous
"""Trainium2 Bass kernel for nn_CoreferenceResolver (coref UNet + pair decoder).

Sharding: core c handles batch b=c//2 and pair-half h=c%2 (496 of 992 pairs).
The gather/cosine/UNet stages are replicated on the two cores sharing a batch;
the extractor linears and group-bilinear decoder are sharded over pairs.

v1 design notes (vs the f32r baseline):
- all weights/activations bf16 (halves DMA bytes, PE still 1 cycle/row)
- fin 1x1 conv + amap gathers + W2 matmuls folded into host-precomputed
  W2' = fin_w^T @ head_w[768:] and a single d1 gather
- extractor = one stacked K=128 matmul per (k, extractor):
  mov rows 0:64 d1[pairs], 64:96 one-hot(hi) (host), 96:128 one-hot(ti)
- enc1 via 3 column-shifted padded images (K=3 matmuls, 6 total)
- weights arrive as a few packed DMA chunks ordered just-in-time so the
  cos->image DMA never queues behind megabytes of weight traffic
- PE warmup chain holds the p-state ramp so real matmuls price at full speed
"""
import os
import sys

for _p in ("/opt/trn_rl_repo",):
    if os.path.isdir(_p) and _p not in sys.path:
        sys.path.insert(0, _p)

import numpy as np

import concourse.bass as bass
import concourse.tile as tile
from concourse import bacc, mybir
from concourse.bass_utils import run_bass_kernel_spmd

f32 = mybir.dt.float32
i16 = mybir.dt.int16
bf16 = mybir.dt.bfloat16
AF = mybir.ActivationFunctionType
OP = mybir.AluOpType

B, L, D, H = 4, 1024, 768, 12
NE, P = 32, 992
BLOCK = 64
G = D // BLOCK          # 12 groups
OUT_CH = 256
NCORES = 8
NH = P // 2             # 496 pairs per core
KD = D // 128           # 6 chunks of the D dim

# ---------------------------------------------------------------------------
# packed-chunk layouts (shapes only; shared by build_nc and pack_inputs)
# entries: name -> (row0, rows, col0, cols); each chunk = one DRAM tensor.
# ---------------------------------------------------------------------------


def _mklayout(rows, entries):
    lay, col = {}, 0
    for name, r0, r, c in entries:
        lay[name] = (r0, r, col, c)
        col += c
    return lay, col


LAY_A64, NC_A64 = _mklayout(64, [
    ("ident", 0, 32, 32),
    ("enc1w3", 0, 3, 192),          # [dx, dy*64+c]
    ("enc2w", 0, 64, 1152),         # [64, 9*128]
    ("dec1wb", 0, 64, 576),         # [64, 9*64]
    ("ag1wx", 0, 64, 64),
    ("ag1ps", 0, 64, 1),
])
LAY_BOTT, NC_BOTT = _mklayout(128, [("bott", 0, 128, 2304)])   # [128, 9*256]
LAY_AG2, NC_AG2 = _mklayout(128, [
    ("ag2wg", 0, 128, 256),         # [128, 2*128]
    ("ag2wx", 0, 128, 128),
    ("ag2ps", 0, 128, 1),
])
LAY_DEC2A, NC_DEC2A = _mklayout(128, [("dec2a", 0, 128, 2304)])  # kc 0:2
LAY_DEC2B, NC_DEC2B = _mklayout(128, [("dec2b", 0, 128, 1152)])  # kc 2
LAY_B128, NC_B128 = _mklayout(128, [
    ("dec1wa", 0, 128, 576),        # [128, 9*64]
    ("ag1wg", 0, 128, 64),
])
LAY_F, NC_F = _mklayout(128, [
    ("stk_h", 0, 128, 768),         # rows 0:64 W2h'; 64:96 <- EW1 (device)
    ("stk_t", 0, 128, 768),         # rows 0:64 W2t'; 64:96 <- EW1t (device)
    ("wdecA", 0, 128, 768),         # blockdiag per k-chunk, o=0
    ("wdecB", 0, 128, 768),         # blockdiag per k-chunk, o=1
    ("ssum", 0, 128, 4),            # [all-ones|0 ; 0|all-ones] selectors
])
LAY_MF, NC_MF = _mklayout(128, [
    ("emask", 0, 32, 1),
    ("enc1b", 0, 64, 1),
    ("enc2b", 0, 128, 1),
    ("bottb", 0, 128, 2),
    ("dec2b_b", 0, 128, 1),
    ("dec1b", 0, 64, 1),
    ("hbp", 0, 128, 6),
    ("tbp", 0, 128, 6),
    ("decb", 0, 2, 1),
])


def build_nc():
    nc = bacc.Bacc("TRN2", target_bir_lowering=False, debug=False, num_devices=NCORES)

    def inp(name, shape, dt=f32):
        return nc.dram_tensor(name, shape, dt, kind="ExternalInput")

    x_b = inp("x_b", [L, D])
    eidx_d = inp("eidx", [128, 8], i16)
    pidx_d = inp("pidx", [128, NH // 16], i16)
    ohm_d = inp("ohm", [64, NH], bf16)
    mf_d = inp("mf", [128, NC_MF])
    a64_d = inp("a64", [64, NC_A64], bf16)
    bott_d = inp("bott_c", [128, NC_BOTT], bf16)
    ag2_d = inp("ag2_c", [128, NC_AG2], bf16)
    dec2a_d = inp("dec2a_c", [128, NC_DEC2A], bf16)
    dec2b_d = inp("dec2b_c", [128, NC_DEC2B], bf16)
    b128_d = inp("b128", [128, NC_B128], bf16)
    w1h_d = inp("w1h", [128, KD * D], bf16)
    w1t_d = inp("w1t", [128, KD * D], bf16)
    f_d = inp("f_c", [128, NC_F], bf16)

    y = nc.dram_tensor("y", [2, NH], f32, kind="ExternalOutput")

    from contextlib import ExitStack
    with tile.TileContext(nc) as tc, ExitStack() as _ctx:
        sbw = _ctx.enter_context(tc.tile_pool(name="sbw", bufs=1))   # persistent
        sbt = _ctx.enter_context(tc.tile_pool(name="sbt", bufs=3))   # rotating temps
        pu_cm = tc.tile_pool(name="pu", bufs=3, space="PSUM")
        pu = pu_cm.__enter__()

        # ---------------- t0: gpsimd: eidx, gather, warm tile, memsets -----
        t_eidx = sbw.tile([128, 8], i16, tag="eidx")
        nc.gpsimd.dma_start(t_eidx[:], eidx_d[:])
        warm = sbw.tile([1, 512], bf16, tag="warm")
        nc.vector.memset(warm[:], 0.0)
        # entities replicated on partition blocks 0:32 / 32:64 / 64:96 / 96:128
        ent_raw = sbw.tile([128, 1, D], f32, tag="entraw")
        nc.gpsimd.dma_gather(ent_raw[:], x_b[:], t_eidx[:],
                             num_idxs=128, num_idxs_reg=128, elem_size=D)
        ent = ent_raw[0:NE, 0, :]

        # padded intermediates (bf16) + border-only memsets
        img3 = sbw.tile([3, 34 * 34], bf16, tag="img3")
        c1p = sbw.tile([64, 34 * 34], bf16, tag="c1p")
        p1p = sbw.tile([64, 18 * 18], bf16, tag="p1p")
        c2p = sbw.tile([128, 18 * 18], bf16, tag="c2p")
        p2p = sbw.tile([128, 10 * 10], bf16, tag="p2p")
        u2p0 = sbw.tile([128, 18 * 18], bf16, tag="u2p0")
        u2p1 = sbw.tile([128, 18 * 18], bf16, tag="u2p1")
        att2p = sbw.tile([128, 18 * 18], bf16, tag="att2p")
        u1p = sbw.tile([128, 34 * 34], bf16, tag="u1p")
        att1p = sbw.tile([64, 34 * 34], bf16, tag="att1p")

        ones_bf = sbw.tile([1, 128], bf16, tag="ones")
        nc.vector.memset(ones_bf[:], 1.0)
        img3v0 = img3[:].rearrange("c (h w) -> c h w", h=34, w=34)
        nc.vector.memset(img3v0[:, 0:34:33, :], 0.0)
        nc.vector.memset(img3v0[:, :, 0:2], 0.0)
        nc.vector.memset(img3v0[:, :, 32:34], 0.0)

        def borders(t, n):
            v = t[:].rearrange("c (h w) -> c h w", h=n, w=n)
            nc.vector.memset(v[:, 0:n:n - 1, :], 0.0)
            nc.vector.memset(v[:, :, 0:n:n - 1], 0.0)

        for t, n in ((c1p, 34), (p1p, 18), (c2p, 18), (p2p, 10), (u2p0, 18),
                     (u2p1, 18), (att2p, 18), (u1p, 34), (att1p, 34)):
            borders(t, n)

        # ---------------- sync-engine DMA chunks (just-in-time order) ------
        def load(dram, shape, dt, tag, eng=None):
            t = sbw.tile(shape, dt, tag=tag)
            (eng or nc.sync).dma_start(t[:], dram[:])
            return t

        t_mf = load(mf_d, [128, NC_MF], f32, "mf")
        t_a64 = load(a64_d, [64, NC_A64], bf16, "a64")

        def loadE(dram, shape, dt, tag):
            t = sbw.tile(shape, dt, tag=tag)
            nc.vector.tensor_copy(t[0:1, 0:1], ent_raw[0:1, 0, 0:1])
            nc.sync.dma_start(t[:], dram[:])
            return t

        t_bott = loadE(bott_d, [128, NC_BOTT], bf16, "bott")
        t_ag2 = loadE(ag2_d, [128, NC_AG2], bf16, "ag2")
        t_pidx = load(pidx_d, [128, NH // 16], i16, "pidx")

        mov_h = sbw.tile([96, NH], bf16, tag="movh")
        mov_t = sbw.tile([96, NH], bf16, tag="movt")
        nc.sync.dma_start(mov_h[64:96, :], ohm_d[0:32, :])
        nc.sync.dma_start(mov_t[64:96, :], ohm_d[32:64, :])

        def vw(tile_, lay, name, shape=None):
            row0, r, c0, c = lay[name]
            ap = tile_[row0:row0 + r, c0:c0 + c]
            if shape is not None and len(shape) > 2:
                pat = {3: "p (a b) -> p a b", 4: "p (a b c) -> p a b c"}[len(shape)]
                kw = dict(zip("abc", shape[1:]))
                ap = ap.rearrange(pat, **kw)
            return ap

        t_ident = vw(t_a64, LAY_A64, "ident")
        t_enc1w = vw(t_a64, LAY_A64, "enc1w3", (3, 3, 64))
        t_enc2w = vw(t_a64, LAY_A64, "enc2w", (64, 9, 128))
        t_dec1wb = vw(t_a64, LAY_A64, "dec1wb", (64, 9, 64))
        t_ag1wx = vw(t_a64, LAY_A64, "ag1wx")
        t_ag1ps = vw(t_a64, LAY_A64, "ag1ps")
        t_bottw = vw(t_bott, LAY_BOTT, "bott", (128, 9, 256))
        t_ag2wg = vw(t_ag2, LAY_AG2, "ag2wg", (128, 2, 128))
        t_ag2wx = vw(t_ag2, LAY_AG2, "ag2wx")
        t_ag2ps = vw(t_ag2, LAY_AG2, "ag2ps")

        t_emask = vw(t_mf, LAY_MF, "emask")
        t_enc1b = vw(t_mf, LAY_MF, "enc1b")
        t_enc2b = vw(t_mf, LAY_MF, "enc2b")
        t_bottb = vw(t_mf, LAY_MF, "bottb")
        t_dec2bb = vw(t_mf, LAY_MF, "dec2b_b")
        t_dec1b = vw(t_mf, LAY_MF, "dec1b")
        t_hbp = vw(t_mf, LAY_MF, "hbp")
        t_tbp = vw(t_mf, LAY_MF, "tbp")
        t_decb = vw(t_mf, LAY_MF, "decb")

        # ---------------- PE warmup chain (p-state ramp) -------------------
        # keeps one unbroken PE busy-run so later bursts price at full speed
        pw_cm = tc.tile_pool(name="pw", bufs=1, space="PSUM")
        pw = pw_cm.__enter__()
        p_warm = pw.tile([1, 512], f32, tag="pw")

        def filler(n):
            for _ in range(n):
                nc.tensor.matmul(p_warm[:], warm[0:1, 0:1], warm[:],
                                 start=True, stop=True)

        filler(int(os.environ.get("NWARM", "14")))

        # ---------------- front-end: norms + transposes + cos --------------
        # dummy sqrt at t0 -> the preamble table load covers Sqrt+Square
        dummy = sbw.tile([1, 2], f32, tag="dum")
        nc.scalar.activation(dummy[:, 0:1], warm[0:1, 0:1], AF.Sqrt)
        sq_scr = sbt.tile([128, D], bf16, tag="t")
        ss = sbw.tile([128, 1], f32, tag="ss")
        nc.scalar.activation(sq_scr[0:96, :], ent_raw[0:96, 0, :], AF.Square,
                             accum_out=ss[0:96, :])
        normc = sbw.tile([128, 1], f32, tag="normc")
        nc.scalar.activation(normc[0:96, :], ss[0:96, :], AF.Sqrt)
        # dummy sigmoid -> hoist the 2nd act-table load off the critical path
        nc.scalar.activation(dummy[:, 1:2], ss[0:1, :], AF.Sigmoid)
        nc.vector.tensor_single_scalar(normc[0:96, :], normc[0:96, :], 1e-13, op=OP.max)
        rinv = sbw.tile([NE, 1], f32, tag="rinv")
        nc.vector.reciprocal(rinv[:], normc[0:NE, :])
        nc.vector.tensor_tensor(out=rinv[:], in0=rinv[:], in1=t_emask, op=OP.mult)
        nrm = sbw.tile([NE, D], bf16, tag="nrm")
        nc.vector.tensor_scalar(out=nrm[:], in0=ent, scalar1=rinv[:],
                                scalar2=None, op0=OP.mult)

        nrmT = sbw.tile([128, KD, NE], bf16, tag="nrmT")
        p_T = pu.tile([128, KD * NE], bf16, tag="pu")
        for k in range(KD):
            nc.tensor.transpose(p_T[:, k * NE:(k + 1) * NE],
                                nrm[:, k * 128:(k + 1) * 128], t_ident)
        nc.vector.tensor_copy(nrmT[:], p_T[:].rearrange("p (a b) -> p a b", a=KD))

        p_cos = pu.tile([NE, NE], f32, tag="pu")
        for k in range(KD):
            nc.tensor.matmul(p_cos[:], nrmT[:, k, :], nrmT[:, k, :],
                             start=(k == 0), stop=(k == KD - 1))
        s_cos = sbw.tile([NE, NE], bf16, tag="scos")
        nc.vector.tensor_copy(s_cos[:], p_cos[:])
        filler(int(os.environ.get("NFILL1", "4")))

        # ---------------- image staging: 3 column-shifted padded copies ----
        img3v = img3[:].rearrange("c (h w) -> c h w", h=34, w=34)
        nc.sync.dma_start(img3v[0:1, 1:33, 2:34], s_cos[:])
        nc.scalar.dma_start(img3v[1:2, 1:33, 1:33], s_cos[:])
        nc.gpsimd.dma_start(img3v[2:3, 1:33, 0:32], s_cos[:])

        # remaining weight chunks: a tiny token copy (reads s_cos) makes each
        # chunk DMA wait until the front-end is done with the DMA device
        def loadT(dram, shape, dt, tag):
            t = sbw.tile(shape, dt, tag=tag)
            nc.vector.tensor_copy(t[0:1, 0:1], s_cos[0:1, 0:1])
            nc.sync.dma_start(t[:], dram[:])
            return t

        t_dec2wa = loadT(dec2a_d, [128, NC_DEC2A], bf16, "dec2a")
        t_dec2wa = t_dec2wa[:].rearrange("p (a b c) -> p a b c", a=2, b=9, c=128)
        t_dec2wb = loadT(dec2b_d, [128, NC_DEC2B], bf16, "dec2b")
        t_dec2wb = t_dec2wb[:].rearrange("p (b c) -> p b c", b=9, c=128)
        t_w1h = loadT(w1h_d, [128, KD * D], bf16, "w1h")
        t_w1h = t_w1h[:].rearrange("p (k d) -> p k d", k=KD)
        t_b128 = loadT(b128_d, [128, NC_B128], bf16, "b128")
        t_dec1wa = vw(t_b128, LAY_B128, "dec1wa", (128, 9, 64))
        t_ag1wg = vw(t_b128, LAY_B128, "ag1wg")
        t_w1t = loadT(w1t_d, [128, KD * D], bf16, "w1t")
        t_w1t = t_w1t[:].rearrange("p (k d) -> p k d", k=KD)
        t_f = loadT(f_d, [128, NC_F], bf16, "f")
        stk_h = vw(t_f, LAY_F, "stk_h", (128, KD, 128))
        stk_t = vw(t_f, LAY_F, "stk_t", (128, KD, 128))
        t_wdecA = vw(t_f, LAY_F, "wdecA", (128, KD, 128))
        t_wdecB = vw(t_f, LAY_F, "wdecB", (128, KD, 128))
        t_ssum = vw(t_f, LAY_F, "ssum")

        # ---------------- enc1: 2 halves x 3 dy matmuls (K=3) --------------
        c1pv = c1p[:].rearrange("c (h w) -> c h w", h=34, w=34)
        p_c1 = pu.tile([64, 1024], f32, tag="pu")
        for hh in range(2):
            for dy in range(3):
                rows = slice(dy + 16 * hh, dy + 16 * hh + 16)
                nc.tensor.matmul(p_c1[:, hh * 512:(hh + 1) * 512],
                                 t_enc1w[:, dy, :],
                                 img3v[0:3, rows, 1:33],
                                 start=(dy == 0), stop=(dy == 2))
        filler(int(os.environ.get("NFILL2", "7")))
        for hh in range(2):
            nc.scalar.activation(c1pv[:, 1 + 16 * hh:17 + 16 * hh, 1:33],
                                 p_c1[:, hh * 512:(hh + 1) * 512].rearrange(
                                     "c (h w) -> c h w", h=16, w=32),
                                 AF.Relu, bias=t_enc1b)

        # ---------------- pool1 (gpsimd) -----------------------------------
        p1pv = p1p[:].rearrange("c (h w) -> c h w", h=18, w=18)
        tmp = sbt.tile([64, 16, 16], bf16, tag="t")
        nc.vector.tensor_max(tmp[:], c1pv[:, 1:33:2, 1:33:2], c1pv[:, 1:33:2, 2:34:2])
        nc.vector.tensor_max(tmp[:], tmp[:], c1pv[:, 2:34:2, 1:33:2])
        nc.vector.tensor_max(p1pv[:, 1:17, 1:17], tmp[:], c1pv[:, 2:34:2, 2:34:2])

        # ---------------- enc2: 9 taps K=64 --------------------------------
        p_c2 = pu.tile([128, 256], f32, tag="pu")
        for tap in range(9):
            dy, dx = tap // 3, tap % 3
            nc.tensor.matmul(p_c2[:], t_enc2w[:, tap, :],
                             p1pv[:, dy:dy + 16, dx:dx + 16],
                             start=(tap == 0), stop=(tap == 8))
        filler(int(os.environ.get("NFILL3", "4")))
        c2pv = c2p[:].rearrange("c (h w) -> c h w", h=18, w=18)
        nc.scalar.activation(c2pv[:, 1:17, 1:17],
                             p_c2[:].rearrange("c (h w) -> c h w", h=16, w=16),
                             AF.Relu, bias=t_enc2b)

        # ---------------- pool2 (gpsimd) -----------------------------------
        p2pv = p2p[:].rearrange("c (h w) -> c h w", h=10, w=10)
        tmp2 = sbt.tile([128, 8, 8], bf16, tag="t")
        nc.vector.tensor_max(tmp2[:], c2pv[:, 1:17:2, 1:17:2], c2pv[:, 1:17:2, 2:18:2])
        nc.vector.tensor_max(tmp2[:], tmp2[:], c2pv[:, 2:18:2, 1:17:2])
        nc.vector.tensor_max(p2pv[:, 1:9, 1:9], tmp2[:], c2pv[:, 2:18:2, 2:18:2])

        # ---------------- bottleneck: 9 taps x 2 M-chunks, K=128 -----------
        c3 = []
        for mc in range(2):
            p_c3 = pu.tile([128, 64], f32, tag="pu")
            for tap in range(9):
                dy, dx = tap // 3, tap % 3
                nc.tensor.matmul(p_c3[:], t_bottw[:, tap, mc * 128:(mc + 1) * 128],
                                 p2pv[:, dy:dy + 8, dx:dx + 8],
                                 start=(tap == 0), stop=(tap == 8))
            c3s = sbt.tile([128, 8, 8], bf16, tag=f"c3_{mc}")
            nc.scalar.activation(c3s[:], p_c3[:].rearrange("c (h w) -> c h w", h=8, w=8),
                                 AF.Relu, bias=t_bottb[:, mc:mc + 1])
            c3.append(c3s)

        # ---------------- up2 ----------------------------------------------
        u2p0v = u2p0[:].rearrange("c (h w) -> c h w", h=18, w=18)
        u2p1v = u2p1[:].rearrange("c (h w) -> c h w", h=18, w=18)
        for src, dv in ((c3[0], u2p0v), (c3[1], u2p1v)):
            for i in range(2):
                for j in range(2):
                    nc.vector.tensor_copy(dv[:, 1 + i:17:2, 1 + j:17:2], src[:])

        # ---------------- attention gate 2 + dec2 (interleaved) ------------
        # the 18 u2-taps of dec2 fill the PE while the psi/sigmoid chain of
        # the attention gate bounces between ACT and DVE
        p_a2 = pu.tile([128, 256], f32, tag="pu")
        nc.tensor.matmul(p_a2[:], t_ag2wg[:, 0, :], u2p0v[:, 1:17, 1:17],
                         start=True, stop=False)
        nc.tensor.matmul(p_a2[:], t_ag2wg[:, 1, :], u2p1v[:, 1:17, 1:17],
                         start=False, stop=False)
        nc.tensor.matmul(p_a2[:], t_ag2wx, c2pv[:, 1:17, 1:17],
                         start=False, stop=True)
        p_d2 = pu.tile([128, 256], f32, tag="pu")
        n_mm = 0
        for kc in range(2):
            src = (u2p0v, u2p1v)[kc]
            for tap in range(9):
                dy, dx = tap // 3, tap % 3
                nc.tensor.matmul(p_d2[:], t_dec2wa[:, kc, tap, :],
                                 src[:, dy:dy + 16, dx:dx + 16],
                                 start=(n_mm == 0), stop=False)
                n_mm += 1
        r2 = sbt.tile([128, 256], bf16, tag="t")
        nc.scalar.activation(r2[:], p_a2[:], AF.Relu)
        p_g2 = pu.tile([1, 256], f32, tag="pu")
        nc.tensor.matmul(p_g2[:], t_ag2ps, r2[:])
        a2 = sbt.tile([1, 256], bf16, tag="a2")
        nc.scalar.activation(a2[:], p_g2[:], AF.Sigmoid)
        p_a2b = pu.tile([128, 256], f32, tag="pu")
        nc.tensor.matmul(p_a2b[:], ones_bf[:], a2[:])
        att2pv = att2p[:].rearrange("c (h w) -> c h w", h=18, w=18)
        nc.vector.tensor_mul(att2pv[:, 1:17, 1:17],
                             p_a2b[:].rearrange("c (h w) -> c h w", h=16, w=16),
                             c2pv[:, 1:17, 1:17])
        for tap in range(9):
            dy, dx = tap // 3, tap % 3
            nc.tensor.matmul(p_d2[:], t_dec2wb[:, tap, :],
                             att2pv[:, dy:dy + 16, dx:dx + 16],
                             start=False, stop=(tap == 8))
        d2s = sbt.tile([128, 256], bf16, tag="d2s")
        nc.scalar.activation(d2s[:], p_d2[:], AF.Relu, bias=t_dec2bb)

        # ---------------- up1 ----------------------------------------------
        u1pv = u1p[:].rearrange("c (h w) -> c h w", h=34, w=34)
        d2v = d2s[:].rearrange("c (h w) -> c h w", h=16, w=16)
        for i in range(2):
            for j in range(2):
                nc.vector.tensor_copy(u1pv[:, 1 + i:33:2, 1 + j:33:2], d2v[:])

        # ---------------- attention gate 1 + dec1 + EW (interleaved) -------
        # dec1's u1-taps and the EW premultiplies fill the PE while the
        # psi/sigmoid chain runs; att1-taps close the dec1 groups afterwards
        p_a1 = pu.tile([64, 1024], f32, tag="pu")
        for hh in range(2):
            rows = slice(1 + 16 * hh, 17 + 16 * hh)
            nc.tensor.matmul(p_a1[:, hh * 512:(hh + 1) * 512], t_ag1wx,
                             c1pv[:, rows, 1:33], start=True, stop=False)
            nc.tensor.matmul(p_a1[:, hh * 512:(hh + 1) * 512], t_ag1wg,
                             u1pv[:, rows, 1:33], start=False, stop=True)
        d1 = sbw.tile([64, 1024], f32, tag="d1")
        p_d1 = pu.tile([64, 1024], f32, tag="pu")

        def dec1_taps(hh, wtile, srcv, start):
            cols = slice(hh * 512, (hh + 1) * 512)
            for tap in range(9):
                dy, dx = tap // 3, tap % 3
                rows = slice(dy + 16 * hh, dy + 16 * hh + 16)
                nc.tensor.matmul(p_d1[:, cols], wtile[:, tap, :],
                                 srcv[:, rows, dx:dx + 32],
                                 start=(start and tap == 0),
                                 stop=(not start and tap == 8))

        dec1_taps(0, t_dec1wa, u1pv, True)
        r1 = sbt.tile([64, 1024], bf16, tag="t")
        nc.scalar.activation(r1[:], p_a1[:], AF.Relu)
        p_g1 = pu.tile([1, 1024], f32, tag="pu")
        for hh in range(2):
            nc.tensor.matmul(p_g1[:, hh * 512:(hh + 1) * 512], t_ag1ps,
                             r1[:, hh * 512:(hh + 1) * 512])
        dec1_taps(1, t_dec1wa, u1pv, True)
        a1 = sbt.tile([1, 1024], bf16, tag="a1")
        nc.scalar.activation(a1[:], p_g1[:], AF.Sigmoid)
        p_a1b = pu.tile([64, 1024], f32, tag="pu")
        for hh in range(2):
            nc.tensor.matmul(p_a1b[:, hh * 512:(hh + 1) * 512], ones_bf[:, :64],
                             a1[:, hh * 512:(hh + 1) * 512])
        att1pv = att1p[:].rearrange("c (h w) -> c h w", h=34, w=34)
        nc.vector.tensor_mul(att1pv[:, 1:33, 1:33],
                             p_a1b[:].rearrange("c (h w) -> c h w", h=32, w=32),
                             c1pv[:, 1:33, 1:33])
        dec1_taps(0, t_dec1wb, att1pv, False)
        # EW-head premultiply: (ent @ head_w[:768]) at rows 64:96
        p_ewh = pu.tile([128, D], f32, tag="pu")
        for k in range(KD):
            for n0, n1 in ((0, 512), (512, D)):
                nc.tensor.matmul(p_ewh[64:96, n0:n1], nrmT[:, k, :],
                                 t_w1h[:, k, n0:n1],
                                 start=(k == 0), stop=(k == KD - 1))
        nc.scalar.activation(stk_h[64:96, :, :].rearrange("p a b -> p (a b)"),
                             p_ewh[64:96, :], AF.Copy, scale=normc[64:96, :])
        dec1_taps(1, t_dec1wb, att1pv, False)
        nc.scalar.activation(d1[:], p_d1[:], AF.Relu, bias=t_dec1b)

        p_ewt = pu.tile([128, D], f32, tag="pu")
        for k in range(KD):
            for n0, n1 in ((0, 512), (512, D)):
                nc.tensor.matmul(p_ewt[64:96, n0:n1], nrmT[:, k, :],
                                 t_w1t[:, k, n0:n1],
                                 start=(k == 0), stop=(k == KD - 1))
        nc.scalar.activation(stk_t[64:96, :, :].rearrange("p a b -> p (a b)"),
                             p_ewt[64:96, :], AF.Copy, scale=normc[64:96, :])

        # ---------------- d1 gather -> mov rows 0:64 ------------------------
        d1g = sbt.tile([64, NH], f32, tag="d1g")
        nc.gpsimd.ap_gather(d1g[:].rearrange("c (n o) -> c n o", o=1),
                            d1[:].rearrange("c (n o) -> c n o", o=1), t_pidx[:],
                            channels=64, num_elems=1024, d=1, num_idxs=NH)
        nc.vector.tensor_copy(mov_h[0:64, :], d1g[:])
        nc.vector.tensor_copy(mov_t[0:64, :], d1g[:])

        pw_cm.__exit__(None, None, None)
        pu_cm.__exit__(None, None, None)

        # ---------------- pair features + decoder --------------------------
        hsT = sbw.tile([128, KD, NH], bf16, tag="hsT")
        tsT = sbw.tile([128, KD, NH], bf16, tag="tsT")
        ph_cm = tc.tile_pool(name="ph", bufs=4, space="PSUM")
        ph = ph_cm.__enter__()
        pd_cm = tc.tile_pool(name="pd", bufs=2, space="PSUM")
        pd = pd_cm.__enter__()
        po_cm = tc.tile_pool(name="po", bufs=1, space="PSUM")
        po = po_cm.__enter__()
        p_out = po.tile([2, NH], f32, tag="po")
        for k in range(KD):
            for (stk, mv, bp, dstT) in ((stk_h, mov_h, t_hbp, hsT),
                                        (stk_t, mov_t, t_tbp, tsT)):
                p_hs = ph.tile([128, NH], f32, tag="ph")
                nc.tensor.matmul(p_hs[:], stk[0:96, k, :], mv[:])
                nc.scalar.activation(dstT[:, k, :], p_hs[:],
                                     AF.Tanh, bias=bp[:, k:k + 1])
            for half, wd in ((0, t_wdecA), (1, t_wdecB)):
                p_u = pd.tile([128, NH], f32, tag="pd")
                nc.tensor.matmul(p_u[:], wd[:, k, :], tsT[:, k, :])
                v = sbt.tile([128, NH], bf16, tag="v")
                nc.vector.tensor_mul(v[:], p_u[:], hsT[:, k, :])
                nc.tensor.matmul(p_out[:], t_ssum[:, 2 * half:2 * half + 2], v[:],
                                 start=(k == 0 and half == 0),
                                 stop=(k == KD - 1 and half == 1))
        out_sb = sbt.tile([2, NH], f32, tag="out")
        nc.scalar.activation(out_sb[:], p_out[:], AF.Identity, bias=t_decb)
        nc.sync.dma_start(y[:], out_sb[:])
        po_cm.__exit__(None, None, None)
        pd_cm.__exit__(None, None, None)
        ph_cm.__exit__(None, None, None)

    nc.compile()
    return nc


def _wrap16(idx, n_slots):
    """int16 index layout for gpsimd gathers: wrapped in 16 partitions,
    replicated across the 8 gpsimd cores."""
    out = np.zeros((128, n_slots), np.int16)
    for j, v in enumerate(idx):
        out[np.arange(8) * 16 + j % 16, j // 16] = v
    return out


def _bf(a):
    import ml_dtypes
    return np.asarray(a, np.float32).astype(ml_dtypes.bfloat16)


def _fill(lay, ncols, rows, dtype, vals):
    out = np.zeros((rows, ncols), dtype=dtype)
    for name, arr in vals.items():
        r0, r, c0, c = lay[name]
        a = np.asarray(arr)
        if a.ndim != 2:
            a = a.reshape(r, c)
        out[r0:r0 + a.shape[0], c0:c0 + a.shape[1]] = a
    return out


def pack_inputs(inputs):
    import ml_dtypes
    bfd = ml_dtypes.bfloat16
    x = np.asarray(inputs["x"], np.float32)
    entity_pos = np.asarray(inputs["entity_pos"])
    hts = np.asarray(inputs["hts"])

    def W(name):
        return np.asarray(inputs[name], np.float32)

    head_w, tail_w = W("head_w"), W("tail_w")
    fin_w = W("fin_w").reshape(OUT_CH, 64)
    fin_b = W("fin_b")
    w2h_f = fin_w.T @ head_w[D:]          # [64, 768]
    w2t_f = fin_w.T @ tail_w[D:]
    hb_f = W("head_b") + fin_b @ head_w[D:]
    tb_f = W("tail_b") + fin_b @ tail_w[D:]

    a64 = _fill(LAY_A64, NC_A64, 64, bfd, {
        "ident": _bf(np.eye(NE)),
        "enc1w3": _bf(W("enc1_w").reshape(64, 3, 3).transpose(2, 1, 0).reshape(3, 192)),
        "enc2w": _bf(W("enc2_w").reshape(128, 64, 9).transpose(1, 2, 0).reshape(64, 1152)),
        "dec1wb": _bf(W("dec1_w").reshape(64, 192, 9).transpose(1, 2, 0)[128:].reshape(64, 576)),
        "ag1wx": _bf(W("ag1_wx").reshape(64, 64).T),
        "ag1ps": _bf(W("ag1_psi").reshape(1, 64).T),
    })
    bott_c = _fill(LAY_BOTT, NC_BOTT, 128, bfd, {
        "bott": _bf(W("bott_w").reshape(256, 128, 9).transpose(1, 2, 0).reshape(128, 2304)),
    })
    ag2_c = _fill(LAY_AG2, NC_AG2, 128, bfd, {
        "ag2wg": _bf(W("ag2_wg").reshape(128, 256).T.reshape(2, 128, 128)
                     .transpose(1, 0, 2).reshape(128, 256)),
        "ag2wx": _bf(W("ag2_wx").reshape(128, 128).T),
        "ag2ps": _bf(W("ag2_psi").reshape(1, 128).T),
    })
    d2w = W("dec2_w").reshape(128, 384, 9).transpose(1, 2, 0)\
        .reshape(3, 128, 9, 128).transpose(1, 0, 2, 3)       # [128, kc, 9, 128]
    dec2a_c = _fill(LAY_DEC2A, NC_DEC2A, 128, bfd,
                    {"dec2a": _bf(d2w[:, 0:2].reshape(128, 2304))})
    dec2b_c = _fill(LAY_DEC2B, NC_DEC2B, 128, bfd,
                    {"dec2b": _bf(d2w[:, 2].reshape(128, 1152))})
    b128 = _fill(LAY_B128, NC_B128, 128, bfd, {
        "dec1wa": _bf(W("dec1_w").reshape(64, 192, 9).transpose(1, 2, 0)[:128]
                      .reshape(128, 576)),
        "ag1wg": _bf(W("ag1_wg").reshape(64, 128).T),
    })
    w1h = _bf(head_w[:D].reshape(KD, 128, D).transpose(1, 0, 2).reshape(128, KD * D))
    w1t = _bf(tail_w[:D].reshape(KD, 128, D).transpose(1, 0, 2).reshape(128, KD * D))

    stk_h = np.zeros((128, KD * 128), np.float32)
    stk_h[0:64] = w2h_f.reshape(64, KD, 128).reshape(64, KD * 128)
    stk_t = np.zeros((128, KD * 128), np.float32)
    stk_t[0:64] = w2t_f.reshape(64, KD, 128).reshape(64, KD * 128)
    wdq = W("decoder_w").reshape(G, 64, 64, 2)        # [g, i, j, o]
    wdA = np.zeros((128, KD, 128), np.float32)
    wdB = np.zeros((128, KD, 128), np.float32)
    for k in range(KD):
        wdA[0:64, k, 0:64] = wdq[2 * k, :, :, 0].T
        wdA[64:128, k, 64:128] = wdq[2 * k + 1, :, :, 0].T
        wdB[0:64, k, 0:64] = wdq[2 * k, :, :, 1].T
        wdB[64:128, k, 64:128] = wdq[2 * k + 1, :, :, 1].T
    ssum = np.zeros((128, 4), np.float32)
    ssum[:, 0] = 1.0
    ssum[:, 3] = 1.0
    f_c = _fill(LAY_F, NC_F, 128, bfd, {
        "stk_h": _bf(stk_h),
        "stk_t": _bf(stk_t),
        "wdecA": _bf(wdA.reshape(128, KD * 128)),
        "wdecB": _bf(wdB.reshape(128, KD * 128)),
        "ssum": _bf(ssum),
    })

    mf = _fill(LAY_MF, NC_MF, 128, np.float32, {
        "enc1b": W("enc1_b").reshape(64, 1),
        "enc2b": W("enc2_b").reshape(128, 1),
        "bottb": W("bott_b").reshape(2, 128).T,
        "dec2b_b": W("dec2_b").reshape(128, 1),
        "dec1b": W("dec1_b").reshape(64, 1),
        "hbp": hb_f.reshape(KD, 128).T,
        "tbp": tb_f.reshape(KD, 128).T,
        "decb": W("decoder_b").reshape(2, 1),
        # emask filled per-core below
    })

    shared = dict(a64=a64, bott_c=bott_c, ag2_c=ag2_c, dec2a_c=dec2a_c,
                  dec2b_c=dec2b_c, b128=b128, w1h=w1h, w1t=w1t, f_c=f_c)

    in_maps = []
    for c in range(NCORES):
        b, h = c // 2, c % 2
        m = dict(shared)
        m["x_b"] = np.ascontiguousarray(x[b])
        start = entity_pos[b, :, 0].astype(np.int64)
        idx = np.minimum(start + 1, L - 1).astype(np.int16)
        m["eidx"] = _wrap16(np.tile(idx, 4), 8)
        mfc = mf.copy()
        r0, r, c0, cc = LAY_MF["emask"]
        mfc[r0:r0 + NE, c0] = (start + 1 < L).astype(np.float32)
        m["mf"] = mfc
        hi = hts[b, h * NH:(h + 1) * NH, 0].astype(np.int64)
        ti = hts[b, h * NH:(h + 1) * NH, 1].astype(np.int64)
        ohm = np.zeros((64, NH), np.float32)
        ohm[hi, np.arange(NH)] = 1.0
        ohm[32 + ti, np.arange(NH)] = 1.0
        m["ohm"] = ohm.astype(bfd)
        m["pidx"] = _wrap16((hi * NE + ti).astype(np.int16), NH // 16)
        in_maps.append(m)
    return in_maps


_NC_CACHE = None


def get_nc():
    global _NC_CACHE
    if _NC_CACHE is None:
        _NC_CACHE = build_nc()
    return _NC_CACHE


def kernel(**inputs):
    nc = get_nc()
    in_maps = pack_inputs(inputs)
    res = run_bass_kernel_spmd(nc, in_maps, core_ids=list(range(NCORES)))
    out = np.empty((B * P, 2), np.float32)
    for c in range(NCORES):
        b, h = c // 2, c % 2
        yc = res.results[c]["y"]                  # [2, NH]
        out[b * P + h * NH:b * P + (h + 1) * NH, :] = yc.T
    return out
